# revision 1
# baseline (speedup 1.0000x reference)
"""GAT message-passing kernel for Trainium2, 8 NeuronCores, dst-partitioned.

Strategy (self-contained; sized for N=50000, E=800000, D=128, H=4, C=16,
ED=64 but parameterized so a tiny config can run in CoreSim):
 - Fold attention vectors into the linear weights on host (tiny matmuls):
   a_src = x @ u_src.T, a_dst = x @ u_dst.T, a_edge = edge_attr @ v.T.
 - Softmax over incoming edges is computed WITHOUT max-subtraction (logits
   are bounded so exp cannot overflow; softmax is shift-invariant) so only
   segment-SUMS are needed, which map onto TensorE one-hot matmuls.
 - Host packs destination nodes into 128-node windows balanced by in-degree
   (LPT), orders/pads edges by window, and ships per-core index arrays.
   Each core owns NWL windows; attention, softmax and aggregation for a
   window happen entirely locally; no collectives are needed.
 - Device computes the node table [xh | a_src | a_dst] from host-transposed
   x, stores it in DRAM (512B rows), then fetches per-edge rows with
   dma_gather (int16 indices; the table is split at row SPLIT so both
   halves are int16-indexable; per-window edge slots are grouped into a
   fixed number of low/high 128-edge blocks, padded with dummy rows whose
   a_src = -1e4 so padded edges contribute exp(-...) = 0).
 - a_dst rows are window-local, fetched from a per-core self-table copied
   once from the global table using the partition-id register.
 - Per 128-edge block a one-hot S = (iota == dstloc) matrix is built on
   VectorE and a single TensorE matmul accumulates [messages | exp(alpha) |
   a_edge] into PSUM per window; self-loops (PyG GATConv default: loop
   edge_attr = per-dst mean of incoming edge_attr) fold in at window close.
"""

import math

import numpy as np

NCORES = 8
D_IN = 128
H_HEADS = 4
C_OUT = 16
HC = H_HEADS * C_OUT  # 64
ED_DIM = 64
NEG_SLOPE = 0.2
DUMMY_ASRC = -1.0e4  # kills padded edges: lrelu -> -2e3, exp -> 0 in f32
TW = 128             # table row width (f32) -> 512B rows for dma_gather

P = 128  # partitions / window node count

TRACE = False       # set by test harness to capture an NTFF profile
LAST_RESULT = None  # BassKernelResults of the last traced run


class _Cfg:
    def __init__(self, nwl, kl, kh, nt_pad, nslots, split):
        self.NWL = nwl            # windows per core
        self.KL = kl              # low-half edge blocks per window
        self.KH = kh              # high-half edge blocks per window
        self.K = kl + kh          # 128-edge blocks per window
        self.NT_PAD = nt_pad      # node-table rows (padded, incl dummy)
        self.NSLOTS = nslots      # NCORES*NWL*128 window-space node slots
        self.SPLIT = split        # table row where the high half starts
        self.ECB = nwl * self.K   # edge blocks per core

    def key(self):
        return (self.NWL, self.KL, self.KH, self.NT_PAD, self.NSLOTS,
                self.SPLIT)


def _fold_weights(W, W_edge, att_src, att_dst, att_edge):
    H, C = att_src.shape
    D = W.shape[1]
    ED = W_edge.shape[1]
    u_src = np.einsum("hc,hcd->hd", att_src, W.reshape(H, C, D))
    u_dst = np.einsum("hc,hcd->hd", att_dst, W.reshape(H, C, D))
    v = np.einsum("hc,hcd->hd", att_edge, W_edge.reshape(H, C, ED))
    # WallT columns = [W.T | u_src.T | u_dst.T | zero pad to TW]
    WallT = np.zeros((D, TW), np.float32)
    WallT[:, :HC] = W.T
    WallT[:, HC:HC + H] = u_src.T
    WallT[:, HC + H:HC + 2 * H] = u_dst.T
    # vT8: rows 0:ED -> [v.T | 0], rows ED:2ED -> [0 | v.T]  (paired matmul)
    vT8 = np.zeros((2 * ED, 2 * H), np.float32)
    vT8[:ED, :H] = v.T
    vT8[ED:, H:] = v.T
    return WallT, vT8


def _partition_nodes(dst, n_nodes, n_windows, reserved):
    """LPT-pack nodes into n_windows bins (<=128 nodes each; bins listed in
    `reserved` hold one fewer), balancing in-degree sums."""
    import heapq

    deg = np.bincount(dst, minlength=n_nodes).astype(np.int64)
    order = np.argsort(-deg, kind="stable")
    cap = np.full(n_windows, P, np.int32)
    for w in reserved:
        cap[w] = P - 1
    heap = [(0, w) for w in range(n_windows)]
    heapq.heapify(heap)
    win_of = np.empty(n_nodes, np.int32)
    slot_of = np.empty(n_nodes, np.int32)
    nodes_in = np.zeros(n_windows, np.int32)
    edges_in = np.zeros(n_windows, np.int64)
    for n in order:
        while True:
            e, w = heapq.heappop(heap)
            if nodes_in[w] < cap[w]:
                break  # full windows are dropped from the heap for good
        win_of[n] = w
        slot_of[n] = nodes_in[w]
        nodes_in[w] += 1
        edges_in[w] += deg[n]
        if nodes_in[w] < cap[w]:
            heapq.heappush(heap, (int(edges_in[w]), w))
    return win_of, slot_of


def _wrap16(idx, num):
    """int16 index array -> dma_gather layout: item i lives at partition
    i%16, col i//16; replicated down the remaining 112 partitions."""
    a = idx.astype(np.int16).reshape(num // 16, 16).T  # [16, num//16]
    return np.ascontiguousarray(np.tile(a, (8, 1)))


def _prep(x, src, dst, edge_attr, WallT, vT8):
    """Build per-core input maps + meta for unsharding."""
    n = x.shape[0]
    nwl = math.ceil(n / (P * NCORES))
    n_windows = NCORES * nwl
    nslots = n_windows * P
    nt_pad = ((nslots + 1 + P - 1) // P) * P
    # table split: both halves must be int16-indexable
    lo_bound = math.ceil(max(0, nt_pad - 32767) / P) * P
    split = max(lo_bound, (min(32767, nslots // 2) // P) * P)
    assert split <= 32768 and nt_pad - split <= 32767

    # reserve window-0 slot 127 as the low-half dummy row
    win_of, slot_of = _partition_nodes(dst, n, n_windows, reserved=[0])
    R_LO = P - 1
    DUMMY = nslots  # high-half dummy row

    winpos = win_of.astype(np.int64) * P + slot_of

    ewin = win_of[dst]
    srow = winpos[src]
    is_low = srow < split

    # fixed per-window low/high block counts across all cores (SPMD)
    nlow = np.bincount(ewin[is_low], minlength=n_windows)
    nhigh = np.bincount(ewin[~is_low], minlength=n_windows)
    kl = max(1, math.ceil(nlow.max() / P))
    kh = max(1, math.ceil(nhigh.max() / P))
    if (kl + kh) % 2:
        kh += 1  # keep total block count even for paired phase-E matmuls
    cfg = _Cfg(nwl, kl, kh, nt_pad, nslots, split)
    K = cfg.K
    epw = K * P

    # ---- place edges: window-major [low | pad | high | pad] ----
    grp = ewin.astype(np.int64) * 2 + (~is_low)
    order_e = np.argsort(grp, kind="stable")
    grp_s = grp[order_e]
    counts = np.bincount(grp_s, minlength=2 * n_windows)
    offs = np.zeros(2 * n_windows + 1, np.int64)
    np.cumsum(counts, out=offs[1:])
    pos = np.arange(len(order_e), dtype=np.int64) - offs[grp_s]
    q = (grp_s // 2) * epw + (grp_s % 2) * (kl * P) + pos

    Q = n_windows * epw
    lowmask_q = (np.arange(Q) % epw) < kl * P
    gsrc_q = np.where(lowmask_q, np.int64(R_LO), np.int64(DUMMY))
    dstloc_q = np.zeros(Q, np.float32)
    gsrc_q[q] = srow[order_e]
    dstloc_q[q] = slot_of[dst[order_e]].astype(np.float32)

    ea_q = np.zeros((Q, ED_DIM), np.float32)
    ea_q[q] = edge_attr[order_e]

    # window-space node features (zero for empty slots)
    x_ws = np.zeros((nslots, D_IN), np.float32)
    x_ws[winpos] = x
    xT = np.zeros((D_IN, nt_pad), np.float32)
    xT[:, :nslots] = x_ws.T

    invcnt_ws = np.ones(nslots, np.float32)
    cnt = np.bincount(dst, minlength=n).astype(np.float32)
    invcnt_ws[winpos] = 1.0 / np.maximum(cnt, 1.0)

    glow_q = np.where(lowmask_q, gsrc_q, 0)
    ghigh_q = np.where(lowmask_q, 0, gsrc_q - split)
    assert glow_q.max() < split and glow_q.min() >= 0
    assert ghigh_q.max() < 32768 and ghigh_q.min() >= 0
    dstwin_q = np.repeat(np.arange(n_windows, dtype=np.int64), epw)
    gdst_q = dstwin_q * P + dstloc_q.astype(np.int64)

    in_maps = []
    pcr = nwl * P
    for c in range(NCORES):
        qs, qe = c * nwl * epw, (c + 1) * nwl * epw
        eac = ea_q[qs:qe].reshape(nwl * K // 2, 2, P, ED_DIM)
        eaT2 = np.ascontiguousarray(
            eac.transpose(1, 3, 0, 2).reshape(2 * ED_DIM, -1))
        dstloc_c = np.ascontiguousarray(
            dstloc_q[qs:qe].reshape(nwl * K, P).T.astype(np.float32))
        lo = glow_q[qs:qe].reshape(nwl, epw)
        hi = ghigh_q[qs:qe].reshape(nwl, epw)
        gd = gdst_q[qs:qe].reshape(nwl, epw) - c * pcr
        assert gd.min() >= 0 and gd.max() < pcr
        glo16 = np.concatenate(
            [_wrap16(lo[w, :kl * P], kl * P) for w in range(nwl)], axis=1)
        ghi16 = np.concatenate(
            [_wrap16(hi[w, kl * P:], kh * P) for w in range(nwl)], axis=1)
        gdst16 = np.concatenate(
            [_wrap16(gd[w], epw) for w in range(nwl)], axis=1)
        invcnt_c = np.ascontiguousarray(
            invcnt_ws[c * pcr:(c + 1) * pcr].reshape(nwl, P).T
            .astype(np.float32))
        in_maps.append(dict(
            xT=xT, eaT2=eaT2, dstloc=dstloc_c, invcnt=invcnt_c,
            glo16=glo16, ghi16=ghi16, gdst16=gdst16,
            WallT=WallT, vT8=vT8,
        ))
    meta = dict(winpos=winpos, cfg=cfg)
    return cfg, in_maps, meta


def _build_nc(cfg):
    import concourse.bass as bass
    import concourse.tile as tile
    from concourse import bacc, mybir
    from contextlib import ExitStack

    f32 = mybir.dt.float32
    i16 = mybir.dt.int16
    NWL, KL, KH, K = cfg.NWL, cfg.KL, cfg.KH, cfg.K
    NT_PAD, NSLOTS, SPLIT = cfg.NT_PAD, cfg.NSLOTS, cfg.SPLIT
    ECB = cfg.ECB
    PCR = NWL * P
    R_LO = P - 1

    nc = bacc.Bacc("TRN2", target_bir_lowering=False, debug=False,
                   num_devices=NCORES)
    xT = nc.dram_tensor("xT", [D_IN, NT_PAD], f32, kind="ExternalInput").ap()
    WallT = nc.dram_tensor("WallT", [D_IN, TW], f32, kind="ExternalInput").ap()
    vT8 = nc.dram_tensor("vT8", [2 * ED_DIM, 2 * H_HEADS], f32,
                         kind="ExternalInput").ap()
    eaT2 = nc.dram_tensor("eaT2", [2 * ED_DIM, ECB * P // 2], f32,
                          kind="ExternalInput").ap()
    dstloc = nc.dram_tensor("dstloc", [P, ECB], f32, kind="ExternalInput").ap()
    invcnt = nc.dram_tensor("invcnt", [P, NWL], f32, kind="ExternalInput").ap()
    glo16 = nc.dram_tensor("glo16", [P, NWL * KL * 8], i16,
                           kind="ExternalInput").ap()
    ghi16 = nc.dram_tensor("ghi16", [P, NWL * KH * 8], i16,
                           kind="ExternalInput").ap()
    gdst16 = nc.dram_tensor("gdst16", [P, NWL * K * 8], i16,
                            kind="ExternalInput").ap()
    out = nc.dram_tensor("out", [PCR, HC], f32, kind="ExternalOutput").ap()
    tableA = nc.dram_tensor("tableA", [NT_PAD, TW], f32).ap()
    selfT = nc.dram_tensor("selfT", [PCR, TW], f32).ap()

    with tile.TileContext(nc) as tc, ExitStack() as ctx:
        cpool = ctx.enter_context(tc.tile_pool(name="const", bufs=1))
        xpool = ctx.enter_context(tc.tile_pool(name="xload", bufs=3))
        tabpool = ctx.enter_context(tc.tile_pool(name="tab", bufs=3))
        eapool = ctx.enter_context(tc.tile_pool(name="ea", bufs=3))
        gpool = ctx.enter_context(tc.tile_pool(name="gather", bufs=2))
        spool = ctx.enter_context(tc.tile_pool(name="onehot", bufs=4))
        wpool = ctx.enter_context(tc.tile_pool(name="work", bufs=3))
        opool = ctx.enter_context(tc.tile_pool(name="outw", bufs=3))
        pst = ctx.enter_context(tc.tile_pool(name="ps_t", bufs=1, space="PSUM"))
        pse = ctx.enter_context(tc.tile_pool(name="ps_e", bufs=1, space="PSUM"))
        psa = ctx.enter_context(tc.tile_pool(name="ps_a", bufs=2, space="PSUM"))
        pstt = ctx.enter_context(tc.tile_pool(name="ps_st", bufs=2, space="PSUM"))
        psad = ctx.enter_context(tc.tile_pool(name="ps_ad", bufs=2, space="PSUM"))
        s2pool = ctx.enter_context(tc.tile_pool(name="sflight", bufs=K + 2))

        # ---- constants ----
        WallT_sb = cpool.tile([P, TW], f32)
        nc.sync.dma_start(WallT_sb[:], WallT[:])
        vT8_sb = cpool.tile([2 * ED_DIM, 2 * H_HEADS], f32)
        nc.sync.dma_start(vT8_sb[:], vT8[:])
        from concourse.masks import make_identity
        ident_sb = cpool.tile([P, P], f32)
        make_identity(nc, ident_sb[:])
        iota_sb = cpool.tile([P, P], f32)
        nc.gpsimd.iota(iota_sb[:], pattern=[[1, P]], base=0,
                       channel_multiplier=0,
                       allow_small_or_imprecise_dtypes=True)
        glo_sb = cpool.tile([P, NWL * KL * 8], i16)
        nc.sync.dma_start(glo_sb[:], glo16[:])
        ghi_sb = cpool.tile([P, NWL * KH * 8], i16)
        nc.sync.dma_start(ghi_sb[:], ghi16[:])
        gdst_sb = cpool.tile([P, NWL * K * 8], i16)
        nc.sync.dma_start(gdst_sb[:], gdst16[:])
        dstloc_sb = cpool.tile([P, ECB], f32)
        nc.sync.dma_start(dstloc_sb[:], dstloc[:])
        invcnt_sb = cpool.tile([P, NWL], f32)
        nc.sync.dma_start(invcnt_sb[:], invcnt[:])
        aedge0_sb = cpool.tile([P, ECB * H_HEADS], f32)

        # ---- phase T: node table = [xh | a_src | a_dst | 0 pad] ----
        NTT = NT_PAD // P
        XB = 8
        for g in range(math.ceil(NTT / XB)):
            t0 = g * XB
            nt = min(XB, NTT - t0)
            xt = xpool.tile([P, XB * P], f32, tag="xt")
            nc.sync.dma_start(xt[:, :nt * P], xT[:, t0 * P:(t0 + nt) * P])
            tab = tabpool.tile([P, XB * TW], f32, tag="tab")
            for t in range(nt):
                ps = pst.tile([P, TW], f32)
                nc.tensor.matmul(out=ps[:], lhsT=xt[:, t * P:(t + 1) * P],
                                 rhs=WallT_sb[:], start=True, stop=True)
                nc.vector.tensor_copy(tab[:, t * TW:(t + 1) * TW], ps[:])
            nc.scalar.dma_start(
                out=tableA[t0 * P:(t0 + nt) * P, :]
                .rearrange("(t p) u -> p t u", p=P),
                in_=tab[:, :nt * TW].rearrange("p (t u) -> p t u", u=TW))
        # dummy rows: a_src = DUMMY_ASRC so padded edges contribute nothing
        dumt = wpool.tile([1, 4], f32, tag="dum")
        nc.vector.memset(dumt[:], DUMMY_ASRC)
        nc.scalar.dma_start(out=tableA[NSLOTS:NSLOTS + 1, HC:HC + 4],
                            in_=dumt[:])
        nc.scalar.dma_start(out=tableA[R_LO:R_LO + 1, HC:HC + 4], in_=dumt[:])

        # ---- self-table: this core's own node rows (partition-id offset) ----
        base = nc.partition_id() * PCR
        nc.gpsimd.dma_start(out=selfT[:, :],
                            in_=tableA[bass.ds(base, PCR), :])

        # ---- phase E: a_edge0 = edge_attr @ v.T, paired 128-edge blocks ----
        NPAIR = ECB // 2
        EB = 32
        for ch in range(math.ceil(NPAIR / EB)):
            b0 = ch * EB
            nb = min(EB, NPAIR - b0)
            ea_ch = eapool.tile([2 * ED_DIM, EB * P], f32, tag="ea_ch")
            nc.sync.dma_start(ea_ch[:, :nb * P], eaT2[:, b0 * P:(b0 + nb) * P])
            ps_e = pse.tile([P, EB * 2 * H_HEADS], f32)
            for b in range(nb):
                nc.tensor.matmul(
                    out=ps_e[:, b * 8:(b + 1) * 8],
                    lhsT=ea_ch[:, b * P:(b + 1) * P],
                    rhs=vT8_sb[:], start=True, stop=True)
            nc.vector.tensor_copy(
                aedge0_sb[:, b0 * 8:(b0 + nb) * 8], ps_e[:, :nb * 8])

        # ---- phase B: per-window attention softmax + aggregation ----
        UH = H_HEADS
        for w in range(NWL):
            G = gpool.tile([P, K * TW], f32, tag="G")
            Gv = G[:].rearrange("p (k u) -> p k u", u=TW)
            GB = 6  # blocks (768 indices) per dma_gather; >~768 idx crashes
            for b0 in range(0, KL, GB):
                nb = min(GB, KL - b0)
                nc.gpsimd.dma_gather(
                    out_ap=Gv[:, b0:b0 + nb, :], in_ap=tableA[0:SPLIT, :],
                    idxs_ap=glo_sb[:, (w * KL + b0) * 8:
                                   (w * KL + b0 + nb) * 8],
                    num_idxs=nb * P, num_idxs_reg=nb * P, elem_size=TW,
                    single_packet=False)
            for b0 in range(0, KH, GB):
                nb = min(GB, KH - b0)
                nc.gpsimd.dma_gather(
                    out_ap=Gv[:, KL + b0:KL + b0 + nb, :],
                    in_ap=tableA[SPLIT:NT_PAD, :],
                    idxs_ap=ghi_sb[:, (w * KH + b0) * 8:
                                   (w * KH + b0 + nb) * 8],
                    num_idxs=nb * P, num_idxs_reg=nb * P, elem_size=TW,
                    single_packet=False)
            selfr = wpool.tile([P, HC + 8], f32, tag="selfr")
            nc.sync.dma_start(selfr[:], selfT[w * P:(w + 1) * P, 0:HC + 8])

            # a_dst(dst) per edge = S.T-expansion of this window's own rows
            ps_adst = psad.tile([P, K * UH], f32)
            S_tiles = []
            for j in range(K):
                S = s2pool.tile([P, P], f32, tag="S")
                nc.vector.tensor_scalar(
                    out=S[:], in0=iota_sb[:],
                    scalar1=dstloc_sb[:, w * K + j:w * K + j + 1],
                    scalar2=None, op0=mybir.AluOpType.is_equal)
                S_tiles.append(S)
                st_ps = pstt.tile([P, P], f32)
                nc.tensor.transpose(out=st_ps[:], in_=S[:],
                                    identity=ident_sb[:])
                st_sb = spool.tile([P, P], f32, tag="St")
                nc.vector.tensor_copy(st_sb[:], st_ps[:])
                nc.tensor.matmul(out=ps_adst[:, j * UH:(j + 1) * UH],
                                 lhsT=st_sb[:], rhs=selfr[:, HC + 4:HC + 8],
                                 start=True, stop=True)

            aw = wpool.tile([P, K * UH], f32, tag="aw")
            aw3 = aw[:].rearrange("p (k u) -> p k u", u=UH)
            # alpha = a_src(src) + a_dst(dst) + a_edge
            nc.vector.tensor_tensor(
                out=aw3, in0=Gv[:, :, HC:HC + UH],
                in1=ps_adst[:].rearrange("p (k u) -> p k u", u=UH),
                op=mybir.AluOpType.add)
            nc.vector.tensor_tensor(
                out=aw[:], in0=aw[:],
                in1=aedge0_sb[:, w * K * UH:(w + 1) * K * UH],
                op=mybir.AluOpType.add)
            # lrelu(x) = slope*x + relu((1-slope)*x), then exp
            lrl = wpool.tile([P, K * UH], f32, tag="lrl")
            nc.scalar.activation(lrl[:], aw[:],
                                 mybir.ActivationFunctionType.Relu,
                                 scale=1.0 - NEG_SLOPE)
            nc.vector.scalar_tensor_tensor(
                out=lrl[:], in0=aw[:], scalar=NEG_SLOPE, in1=lrl[:],
                op0=mybir.AluOpType.mult, op1=mybir.AluOpType.add)
            srhs = wpool.tile([P, K * 8], f32, tag="srhs")
            srhs3 = srhs[:].rearrange("p (k u) -> p k u", u=8)
            nc.scalar.activation(srhs3[:, :, 0:4],
                                 lrl[:].rearrange("p (k u) -> p k u", u=4),
                                 mybir.ActivationFunctionType.Exp)
            nc.vector.tensor_copy(
                srhs3[:, :, 4:8],
                aedge0_sb[:, w * K * UH:(w + 1) * K * UH]
                .rearrange("p (k u) -> p k u", u=4))

            # one matmul per block: rhs = [expal*xh | expal | a_edge0]
            ps_agg = psa.tile([P, HC + 8], f32)
            for j in range(K):
                S = S_tiles[j]
                M = spool.tile([P, HC + 8], f32, tag="M")
                expal_b = srhs3[:, j, 0:4].unsqueeze(2).broadcast_to(
                    [P, 4, C_OUT])
                nc.vector.tensor_tensor(
                    out=M[:, 0:HC].rearrange("p (h c) -> p h c", c=C_OUT),
                    in0=Gv[:, j, 0:HC].rearrange("p (h c) -> p h c", c=C_OUT),
                    in1=expal_b, op=mybir.AluOpType.mult)
                nc.vector.tensor_copy(M[:, HC:HC + 8],
                                      srhs[:, j * 8:(j + 1) * 8])
                nc.tensor.matmul(out=ps_agg[:], lhsT=S[:], rhs=M[:],
                                 start=(j == 0), stop=(j == K - 1))

            # ---- window close: self-loop term + normalization ----
            lae = wpool.tile([P, 4], f32, tag="lae")
            nc.vector.tensor_scalar(out=lae[:], in0=ps_agg[:, HC + 4:HC + 8],
                                    scalar1=invcnt_sb[:, w:w + 1],
                                    scalar2=None, op0=mybir.AluOpType.mult)
            asf = wpool.tile([P, 4], f32, tag="asf")
            nc.vector.tensor_tensor(out=asf[:], in0=selfr[:, HC:HC + 4],
                                    in1=selfr[:, HC + 4:HC + 8],
                                    op=mybir.AluOpType.add)
            nc.vector.tensor_tensor(out=asf[:], in0=asf[:], in1=lae[:],
                                    op=mybir.AluOpType.add)
            es = wpool.tile([P, 4], f32, tag="es")
            nc.scalar.activation(es[:], asf[:],
                                 mybir.ActivationFunctionType.Relu,
                                 scale=1.0 - NEG_SLOPE)
            nc.vector.scalar_tensor_tensor(
                out=es[:], in0=asf[:], scalar=NEG_SLOPE, in1=es[:],
                op0=mybir.AluOpType.mult, op1=mybir.AluOpType.add)
            nc.scalar.activation(es[:], es[:],
                                 mybir.ActivationFunctionType.Exp)
            # den = exp(alpha_self) + 1e-30 + sum_edges exp(alpha); the 1e-30
            # keeps the reserved dummy slots finite (den=0 -> NaN otherwise)
            den = wpool.tile([P, 4], f32, tag="den")
            nc.vector.scalar_tensor_tensor(
                out=den[:], in0=es[:], scalar=1e-30,
                in1=ps_agg[:, HC:HC + 4],
                op0=mybir.AluOpType.add, op1=mybir.AluOpType.add)
            rec = wpool.tile([P, 4], f32, tag="rec")
            nc.vector.reciprocal(rec[:], den[:])
            ot = opool.tile([P, HC], f32, tag="ot")
            es_b = es[:].unsqueeze(2).broadcast_to([P, 4, C_OUT])
            nc.vector.tensor_tensor(
                out=ot[:].rearrange("p (h c) -> p h c", c=C_OUT),
                in0=selfr[:, 0:HC].rearrange("p (h c) -> p h c", c=C_OUT),
                in1=es_b, op=mybir.AluOpType.mult)
            nc.vector.tensor_tensor(out=ot[:], in0=ot[:], in1=ps_agg[:, 0:HC],
                                    op=mybir.AluOpType.add)
            rec_b = rec[:].unsqueeze(2).broadcast_to([P, 4, C_OUT])
            nc.vector.tensor_tensor(
                out=ot[:].rearrange("p (h c) -> p h c", c=C_OUT),
                in0=ot[:].rearrange("p (h c) -> p h c", c=C_OUT),
                in1=rec_b, op=mybir.AluOpType.mult)
            nc.sync.dma_start(out[w * P:(w + 1) * P, :], ot[:])

    nc.compile()
    return nc


_NC_CACHE = {}


def _get_nc(cfg):
    k = cfg.key()
    if k not in _NC_CACHE:
        _NC_CACHE[k] = _build_nc(cfg)
    return _NC_CACHE[k]


def kernel(**inputs):
    x = np.asarray(inputs["x"], dtype=np.float32)
    ei = np.asarray(inputs["edge_index"])
    ea = np.asarray(inputs["edge_attr"], dtype=np.float32)
    W = np.asarray(inputs["W"], dtype=np.float32)
    W_edge = np.asarray(inputs["W_edge"], dtype=np.float32)
    att_src = np.asarray(inputs["att_src"], dtype=np.float32)
    att_dst = np.asarray(inputs["att_dst"], dtype=np.float32)
    att_edge = np.asarray(inputs["att_edge"], dtype=np.float32)
    bias = np.asarray(inputs["bias"], dtype=np.float32)

    src = ei[0].astype(np.int64)
    dst = ei[1].astype(np.int64)
    WallT, vT8 = _fold_weights(W, W_edge, att_src, att_dst, att_edge)

    cfg, in_maps, meta = _prep(x, src, dst, ea, WallT, vT8)
    nc = _get_nc(cfg)

    from concourse.bass_utils import run_bass_kernel_spmd
    res = run_bass_kernel_spmd(nc, in_maps, core_ids=list(range(NCORES)),
                               trace=TRACE)
    if TRACE:
        global LAST_RESULT
        LAST_RESULT = res

    out_ws = np.concatenate([res.results[c]["out"] for c in range(NCORES)],
                            axis=0)  # [NSLOTS, HC] in window space
    out = out_ws[meta["winpos"]]
    return (out + bias[None, :]).astype(np.float32)



# revision 6
# speedup vs baseline: 2.6033x; 2.6033x over previous
"""GAT message-passing kernel for Trainium2, 8 NeuronCores, dst-partitioned.

v2 (bf16 + batched one-hots + rotated SWDGE queues):
 - Fold attention vectors into the linear weights on host (tiny matmuls):
   a_src = x @ u_src.T, a_dst = x @ u_dst.T, a_edge = edge_attr @ v.T.
 - Softmax over incoming edges is computed WITHOUT max-subtraction (logits
   are bounded so exp cannot overflow; softmax is shift-invariant) so only
   segment-SUMS are needed, which map onto TensorE one-hot matmuls.
 - Host packs destination nodes into 128-slot windows (slot 127 of every
   window is reserved as a trash slot for padded edges), balanced by
   in-degree (LPT), orders/pads edges by window, ships per-core indices.
 - Everything on device runs in bf16 (tolerance 2e-2; bf16 keeps ~0.4%/elem)
   which makes PE matmuls 4x faster and halves gather bytes.
 - Node table [xh | a_src | a_dst] is bf16 256B rows in DRAM; per-edge rows
   are fetched with dma_gather (int16 idx, split low/high tables so both are
   int16-indexable). Padded slots have idx=-1 which the Q7 kernel trims from
   the tail; their dstloc points at the reserved slot 127 so stale SBUF data
   lands in a discarded output column (G buffers are zeroed once at start so
   stale data is never NaN).
 - Gathers rotate across the 4 SWDGE queues (4 Q7 core pairs + descriptor
   rings) which overlaps descriptor generation with DMA drain: measured
   ~3ns/row vs ~12ns/row on a single queue.
 - Per window, both one-hot matrices are built in ONE vector op each:
   S  [e, n] = (iota_col == dstloc_e)   (agg matmul lhsT, edge on partition)
   St [n, e] = (dstlocT == iota_part)   (a_dst expansion lhsT)
   dstlocT rows are broadcast to all partitions by a stride-0 DMA read.
 - Self-loops (PyG GATConv: loop edge_attr = per-dst mean of incoming
   edge_attr) fold in at window close from the unweighted aedge segment-sum.
"""

import math

import numpy as np

NCORES = 8
D_IN = 128
H_HEADS = 4
C_OUT = 16
HC = H_HEADS * C_OUT  # 64
ED_DIM = 64
NEG_SLOPE = 0.2
TW = 128             # table row width (bf16) -> 256B rows for dma_gather

P = 128  # partitions / window slot count (127 real nodes + trash slot)

TRACE = False       # set by test harness to capture an NTFF profile
LAST_RESULT = None  # BassKernelResults of the last traced run


class _Cfg:
    def __init__(self, nwl, kl, kh, nt_pad, nslots, split):
        self.NWL = nwl            # windows per core
        self.KL = kl              # low-half edge blocks per window
        self.KH = kh              # high-half edge blocks per window
        self.K = kl + kh          # 128-edge blocks per window
        self.NT_PAD = nt_pad      # node-table rows
        self.NSLOTS = nslots      # NCORES*NWL*128 window-space node slots
        self.SPLIT = split        # table row where the high half starts
        self.ECB = nwl * self.K   # edge blocks per core

    def key(self):
        return (self.NWL, self.KL, self.KH, self.NT_PAD, self.NSLOTS,
                self.SPLIT)


def _fold_weights(W, W_edge, att_src, att_dst, att_edge):
    H, C = att_src.shape
    D = W.shape[1]
    ED = W_edge.shape[1]
    u_src = np.einsum("hc,hcd->hd", att_src, W.reshape(H, C, D))
    u_dst = np.einsum("hc,hcd->hd", att_dst, W.reshape(H, C, D))
    v = np.einsum("hc,hcd->hd", att_edge, W_edge.reshape(H, C, ED))
    # WallT columns = [W.T | u_src.T | u_dst.T | zero pad to TW]
    WallT = np.zeros((D, TW), np.float32)
    WallT[:, :HC] = W.T
    WallT[:, HC:HC + H] = u_src.T
    WallT[:, HC + H:HC + 2 * H] = u_dst.T
    # vT8: rows 0:ED -> [v.T | 0], rows ED:2ED -> [0 | v.T]  (paired matmul)
    vT8 = np.zeros((2 * ED, 2 * H), np.float32)
    vT8[:ED, :H] = v.T
    vT8[ED:, H:] = v.T
    return WallT, vT8


def _partition_nodes(dst, n_nodes, n_windows):
    """LPT-pack nodes into n_windows bins of <=127 nodes each (slot 127 is
    reserved), balancing in-degree sums."""
    import heapq

    cap = P - 1
    deg = np.bincount(dst, minlength=n_nodes).astype(np.int64)
    order = np.argsort(-deg, kind="stable")
    heap = [(0, w) for w in range(n_windows)]
    heapq.heapify(heap)
    win_of = np.empty(n_nodes, np.int32)
    slot_of = np.empty(n_nodes, np.int32)
    nodes_in = np.zeros(n_windows, np.int32)
    edges_in = np.zeros(n_windows, np.int64)
    for n in order:
        while True:
            e, w = heapq.heappop(heap)
            if nodes_in[w] < cap:
                break  # full windows are dropped from the heap for good
        win_of[n] = w
        slot_of[n] = nodes_in[w]
        nodes_in[w] += 1
        edges_in[w] += deg[n]
        if nodes_in[w] < cap:
            heapq.heappush(heap, (int(edges_in[w]), w))
    return win_of, slot_of


def _wrap16(idx, num):
    """int16 index array -> dma_gather layout: item i lives at partition
    i%16, col i//16; replicated down the remaining 112 partitions."""
    a = idx.astype(np.int16).reshape(num // 16, 16).T  # [16, num//16]
    return np.ascontiguousarray(np.tile(a, (8, 1)))


def _prep(x, src, dst, edge_attr, WallT, vT8):
    """Build per-core input maps + meta for unsharding."""
    import ml_dtypes
    bf = ml_dtypes.bfloat16

    n = x.shape[0]
    nwl = math.ceil(n / ((P - 1) * NCORES))
    n_windows = NCORES * nwl
    nslots = n_windows * P
    nt_pad = nslots
    split = (nslots // 2 // P) * P
    assert split <= 32767 and nt_pad - split <= 32767

    win_of, slot_of = _partition_nodes(dst, n, n_windows)
    R_TRASH = P - 1

    winpos = win_of.astype(np.int64) * P + slot_of

    ewin = win_of[dst]
    srow = winpos[src]
    is_low = srow < split

    # fixed per-window low/high block counts across all cores (SPMD)
    nlow = np.bincount(ewin[is_low], minlength=n_windows)
    nhigh = np.bincount(ewin[~is_low], minlength=n_windows)
    kl = max(1, math.ceil(nlow.max() / P))
    kh = max(1, math.ceil(nhigh.max() / P))
    if (kl + kh) % 2:
        kh += 1  # keep total block count even for paired phase-E matmuls
    cfg = _Cfg(nwl, kl, kh, nt_pad, nslots, split)
    K = cfg.K
    epw = K * P

    # ---- place edges: window-major [low | pad(-1) | high | pad(-1)] ----
    grp = ewin.astype(np.int64) * 2 + (~is_low)
    order_e = np.argsort(grp, kind="stable")
    grp_s = grp[order_e]
    counts = np.bincount(grp_s, minlength=2 * n_windows)
    offs = np.zeros(2 * n_windows + 1, np.int64)
    np.cumsum(counts, out=offs[1:])
    pos = np.arange(len(order_e), dtype=np.int64) - offs[grp_s]
    q = (grp_s // 2) * epw + (grp_s % 2) * (kl * P) + pos

    Q = n_windows * epw
    lowmask_q = (np.arange(Q) % epw) < kl * P
    gsrc_q = np.full(Q, -1, np.int64)
    dstloc_q = np.full(Q, R_TRASH, np.uint8)
    gsrc_q[q] = srow[order_e]
    dstloc_q[q] = slot_of[dst[order_e]].astype(np.uint8)

    ea_q = np.zeros((Q, ED_DIM), np.float32)
    ea_q[q] = edge_attr[order_e]

    # window-space node features (zero for empty slots)
    x_ws = np.zeros((nslots, D_IN), np.float32)
    x_ws[winpos] = x
    xT = np.zeros((D_IN, nt_pad), bf)
    xT[:, :nslots] = x_ws.T.astype(bf)

    invcnt_ws = np.ones(nslots, np.float32)
    cnt = np.bincount(dst, minlength=n).astype(np.float32)
    invcnt_ws[winpos] = 1.0 / np.maximum(cnt, 1.0)

    PAD_IDX = 0  # -1 enables Q7 tail-trimming; 0 gathers row 0 (debug)
    glow_q = np.where(lowmask_q & (gsrc_q >= 0), gsrc_q, PAD_IDX)
    ghigh_q = np.where(~lowmask_q & (gsrc_q >= 0), gsrc_q - split, PAD_IDX)
    assert glow_q.max() < split and ghigh_q.max() < nt_pad - split

    in_maps = []
    pcr = nwl * P
    WallT16 = WallT.astype(bf)
    vT816 = vT8.astype(bf)
    for c in range(NCORES):
        qs, qe = c * nwl * epw, (c + 1) * nwl * epw
        eac = ea_q[qs:qe].reshape(nwl * K // 2, 2, P, ED_DIM)
        eaT2 = np.ascontiguousarray(
            eac.transpose(1, 3, 0, 2).reshape(2 * ED_DIM, -1)).astype(bf)
        dstloc_c = np.ascontiguousarray(
            dstloc_q[qs:qe].reshape(nwl * K, P).T)  # [P, ECB] u8
        dstlocT_c = np.ascontiguousarray(
            dstloc_q[qs:qe].reshape(nwl, epw))      # [NWL, K*128] u8
        lo = glow_q[qs:qe].reshape(nwl, epw)
        hi = ghigh_q[qs:qe].reshape(nwl, epw)
        glo16 = np.concatenate(
            [_wrap16(lo[w, :kl * P], kl * P) for w in range(nwl)], axis=1)
        ghi16 = np.concatenate(
            [_wrap16(hi[w, kl * P:], kh * P) for w in range(nwl)], axis=1)
        invcnt_c = np.ascontiguousarray(
            invcnt_ws[c * pcr:(c + 1) * pcr].reshape(nwl, P).T
            .astype(np.float32))
        in_maps.append(dict(
            xT=xT, eaT2=eaT2, dstloc=dstloc_c, dstlocT=dstlocT_c,
            invcnt=invcnt_c, glo16=glo16, ghi16=ghi16,
            WallT=WallT16, vT8=vT816,
        ))
    meta = dict(winpos=winpos, cfg=cfg)
    return cfg, in_maps, meta


def _build_nc(cfg):
    import concourse.bass as bass
    import concourse.tile as tile
    from concourse import bacc, mybir
    from contextlib import ExitStack

    f32 = mybir.dt.float32
    bf16 = mybir.dt.bfloat16
    i16 = mybir.dt.int16
    u8 = mybir.dt.uint8
    NWL, KL, KH, K = cfg.NWL, cfg.KL, cfg.KH, cfg.K
    NT_PAD, SPLIT = cfg.NT_PAD, cfg.SPLIT
    ECB = cfg.ECB
    PCR = NWL * P
    UH = H_HEADS

    nc = bacc.Bacc("TRN2", target_bir_lowering=False, debug=False,
                   num_devices=NCORES, num_swdge_queues=4,
                   dynamic_dma_scratch_size=65536)
    xT = nc.dram_tensor("xT", [D_IN, NT_PAD], bf16, kind="ExternalInput").ap()
    WallT = nc.dram_tensor("WallT", [D_IN, TW], bf16,
                           kind="ExternalInput").ap()
    vT8 = nc.dram_tensor("vT8", [2 * ED_DIM, 2 * H_HEADS], bf16,
                         kind="ExternalInput").ap()
    eaT2 = nc.dram_tensor("eaT2", [2 * ED_DIM, ECB * P // 2], bf16,
                          kind="ExternalInput").ap()
    dstloc = nc.dram_tensor("dstloc", [P, ECB], u8, kind="ExternalInput").ap()
    dstlocT = nc.dram_tensor("dstlocT", [NWL, K * P], u8,
                             kind="ExternalInput").ap()
    invcnt = nc.dram_tensor("invcnt", [P, NWL], f32, kind="ExternalInput").ap()
    glo16 = nc.dram_tensor("glo16", [P, NWL * KL * 8], i16,
                           kind="ExternalInput").ap()
    ghi16 = nc.dram_tensor("ghi16", [P, NWL * KH * 8], i16,
                           kind="ExternalInput").ap()
    out = nc.dram_tensor("out", [PCR, HC], f32, kind="ExternalOutput").ap()
    tableA = nc.dram_tensor("tableA", [NT_PAD, TW], bf16).ap()
    selfT = nc.dram_tensor("selfT", [PCR, TW], bf16).ap()

    with tile.TileContext(nc) as tc, ExitStack() as ctx:
        cpool = ctx.enter_context(tc.tile_pool(name="const", bufs=1))
        xpool = ctx.enter_context(tc.tile_pool(name="xload", bufs=3))
        tabpool = ctx.enter_context(tc.tile_pool(name="tab", bufs=3))
        eapool = ctx.enter_context(tc.tile_pool(name="ea", bufs=2))
        gpool = ctx.enter_context(tc.tile_pool(name="gather", bufs=3))
        dtpool = ctx.enter_context(tc.tile_pool(name="dstT", bufs=2))
        onepool = ctx.enter_context(tc.tile_pool(name="onehot", bufs=2))
        mpool = ctx.enter_context(tc.tile_pool(name="msg", bufs=2))
        wpool = ctx.enter_context(tc.tile_pool(name="work", bufs=3))
        opool = ctx.enter_context(tc.tile_pool(name="outw", bufs=3))
        pst = ctx.enter_context(tc.tile_pool(name="ps_t", bufs=2, space="PSUM"))
        pse = ctx.enter_context(tc.tile_pool(name="ps_e", bufs=2, space="PSUM"))
        psa = ctx.enter_context(tc.tile_pool(name="ps_a", bufs=2, space="PSUM"))
        psad = ctx.enter_context(tc.tile_pool(name="ps_ad", bufs=2,
                                              space="PSUM"))

        # ---- constants ----
        WallT_sb = cpool.tile([P, TW], bf16)
        nc.sync.dma_start(WallT_sb[:], WallT[:])
        vT8_sb = cpool.tile([2 * ED_DIM, 2 * H_HEADS], bf16)
        nc.sync.dma_start(vT8_sb[:], vT8[:])
        iota_rep = cpool.tile([P, K * P], u8)  # value = col % 128
        nc.gpsimd.iota(iota_rep[:].rearrange("p (k u) -> p k u", u=P),
                       pattern=[[0, K], [1, P]], base=0,
                       channel_multiplier=0,
                       allow_small_or_imprecise_dtypes=True)
        piota = cpool.tile([P, 1], f32)  # value = partition idx
        nc.gpsimd.iota(piota[:], pattern=[[0, 1]], base=0,
                       channel_multiplier=1,
                       allow_small_or_imprecise_dtypes=True)
        glo_sb = cpool.tile([P, NWL * KL * 8], i16)
        nc.sync.dma_start(glo_sb[:], glo16[:])
        ghi_sb = cpool.tile([P, NWL * KH * 8], i16)
        nc.sync.dma_start(ghi_sb[:], ghi16[:])
        dstloc_sb = cpool.tile([P, ECB], u8)
        nc.sync.dma_start(dstloc_sb[:], dstloc[:])
        invcnt_sb = cpool.tile([P, NWL], f32)
        nc.sync.dma_start(invcnt_sb[:], invcnt[:])

        # zero the gather buffers once: trimmed (padded) slots keep stale
        # data, which must be finite (never NaN from a previous NEFF)
        for _ in range(3):
            G0 = gpool.tile([P, K * TW], bf16, tag="G")
            nc.vector.memset(G0[:], 0.0)

        # ---- phase T: node table = [xh | a_src | a_dst | 0 pad] ----
        NTT = NT_PAD // P
        XB = 8
        for g in range(math.ceil(NTT / XB)):
            t0 = g * XB
            nt = min(XB, NTT - t0)
            xt = xpool.tile([P, XB * P], bf16, tag="xt")
            nc.sync.dma_start(xt[:, :nt * P], xT[:, t0 * P:(t0 + nt) * P])
            tab = tabpool.tile([P, XB * TW], bf16, tag="tab")
            for t in range(nt):
                ps = pst.tile([P, TW], f32)
                nc.tensor.matmul(out=ps[:], lhsT=xt[:, t * P:(t + 1) * P],
                                 rhs=WallT_sb[:], start=True, stop=True)
                if t % 2 == 0:
                    nc.vector.tensor_copy(tab[:, t * TW:(t + 1) * TW], ps[:])
                else:
                    nc.scalar.copy(tab[:, t * TW:(t + 1) * TW], ps[:])
            nc.scalar.dma_start(
                out=tableA[t0 * P:(t0 + nt) * P, :]
                .rearrange("(t p) u -> p t u", p=P),
                in_=tab[:, :nt * TW].rearrange("p (t u) -> p t u", u=TW))

        # ---- self-table: this core's own node rows (partition-id offset) ----
        base = nc.partition_id() * PCR
        nc.gpsimd.dma_start(out=selfT[:, :],
                            in_=tableA[bass.ds(base, PCR), :])

        # ---- phase B: per-window attention softmax + aggregation ----
        for w in range(NWL):
            G = gpool.tile([P, K * TW], bf16, tag="G")
            Gv = G[:].rearrange("p (k u) -> p k u", u=TW)
            nc.gpsimd.dma_gather(
                out_ap=Gv[:, 0:KL, :], in_ap=tableA[0:SPLIT, :],
                idxs_ap=glo_sb[:, w * KL * 8:(w + 1) * KL * 8],
                num_idxs=KL * P, num_idxs_reg=KL * P, elem_size=TW,
                single_packet=False, queue_num=(2 * w) % 4)
            nc.gpsimd.dma_gather(
                out_ap=Gv[:, KL:K, :], in_ap=tableA[SPLIT:NT_PAD, :],
                idxs_ap=ghi_sb[:, w * KH * 8:(w + 1) * KH * 8],
                num_idxs=KH * P, num_idxs_reg=KH * P, elem_size=TW,
                single_packet=False, queue_num=(2 * w + 1) % 4)

            selfr = wpool.tile([P, HC + 8], bf16, tag="selfr")
            nc.sync.dma_start(selfr[:], selfT[w * P:(w + 1) * P, 0:HC + 8])

            # one-hot builds (one vector op each, all K blocks at once)
            dT = dtpool.tile([P, K * P], u8, tag="dT")
            nc.sync.dma_start(dT[:], dstlocT[w:w + 1, :]
                              .broadcast_to([P, K * P]))
            St = onepool.tile([P, K * P], bf16, tag="St")
            nc.vector.tensor_scalar(
                out=St[:], in0=dT[:], scalar1=piota[:], scalar2=None,
                op0=mybir.AluOpType.is_equal)
            S = onepool.tile([P, K * P], bf16, tag="S")
            nc.vector.tensor_tensor(
                out=S[:].rearrange("p (k u) -> p k u", u=P),
                in0=iota_rep[:].rearrange("p (k u) -> p k u", u=P),
                in1=dstloc_sb[:, w * K:(w + 1) * K].unsqueeze(2)
                .broadcast_to([P, K, P]),
                op=mybir.AluOpType.is_equal)

            # a_dst(dst) per edge: St-block matmuls against own a_dst rows
            ps_adst = psad.tile([P, K * UH], f32)
            for j in range(K):
                nc.tensor.matmul(out=ps_adst[:, j * UH:(j + 1) * UH],
                                 lhsT=St[:, j * P:(j + 1) * P],
                                 rhs=selfr[:, HC + 4:HC + 8],
                                 start=True, stop=True)

            # phase E slice for this window: a_edge0 = edge_attr @ v.T
            ea_ch = eapool.tile([2 * ED_DIM, (K // 2) * P], bf16, tag="ea")
            nc.sync.dma_start(
                ea_ch[:], eaT2[:, w * (K // 2) * P:(w + 1) * (K // 2) * P])
            ps_e = pse.tile([P, K * UH], f32)
            for jj in range(K // 2):
                nc.tensor.matmul(
                    out=ps_e[:, jj * 8:(jj + 1) * 8],
                    lhsT=ea_ch[:, jj * P:(jj + 1) * P],
                    rhs=vT8_sb[:], start=True, stop=True)

            # alpha = a_src(src) + a_dst(dst) + a_edge
            aw = wpool.tile([P, K * UH], f32, tag="aw")
            aw3 = aw[:].rearrange("p (k u) -> p k u", u=UH)
            nc.vector.tensor_tensor(
                out=aw3, in0=Gv[:, :, HC:HC + UH],
                in1=ps_adst[:].rearrange("p (k u) -> p k u", u=UH),
                op=mybir.AluOpType.add)
            nc.vector.tensor_tensor(
                out=aw[:], in0=aw[:], in1=ps_e[:], op=mybir.AluOpType.add)
            # lrelu(x) = slope*x + relu((1-slope)*x), then exp
            lrl = wpool.tile([P, K * UH], f32, tag="lrl")
            nc.scalar.activation(lrl[:], aw[:],
                                 mybir.ActivationFunctionType.Relu,
                                 scale=1.0 - NEG_SLOPE)
            nc.vector.scalar_tensor_tensor(
                out=lrl[:], in0=aw[:], scalar=NEG_SLOPE, in1=lrl[:],
                op0=mybir.AluOpType.mult, op1=mybir.AluOpType.add)

            # M = [expal * xh | expal | a_edge0] per block
            M = mpool.tile([P, K * (HC + 8)], bf16, tag="M")
            M3 = M[:].rearrange("p (k u) -> p k u", u=HC + 8)
            nc.scalar.activation(M3[:, :, HC:HC + UH],
                                 lrl[:].rearrange("p (k u) -> p k u", u=UH),
                                 mybir.ActivationFunctionType.Exp)
            nc.vector.tensor_copy(
                M3[:, :, HC + 4:HC + 8],
                ps_e[:].rearrange("p (k u) -> p k u", u=UH))
            expal_b = (M3[:, :, HC:HC + UH].unsqueeze(3)
                       .broadcast_to([P, K, UH, C_OUT]))
            nc.vector.tensor_tensor(
                out=M3[:, :, 0:HC].rearrange("p k (h c) -> p k h c", c=C_OUT),
                in0=Gv[:, :, 0:HC].rearrange("p k (h c) -> p k h c", c=C_OUT),
                in1=expal_b, op=mybir.AluOpType.mult)

            # segment sums: one matmul per block, accumulated in PSUM
            ps_agg = psa.tile([P, HC + 8], f32)
            for j in range(K):
                nc.tensor.matmul(out=ps_agg[:], lhsT=S[:, j * P:(j + 1) * P],
                                 rhs=M[:, j * (HC + 8):(j + 1) * (HC + 8)],
                                 start=(j == 0), stop=(j == K - 1))

            # ---- window close: self-loop term + normalization ----
            lae = wpool.tile([P, 4], f32, tag="lae")
            nc.vector.tensor_scalar(out=lae[:], in0=ps_agg[:, HC + 4:HC + 8],
                                    scalar1=invcnt_sb[:, w:w + 1],
                                    scalar2=None, op0=mybir.AluOpType.mult)
            asf = wpool.tile([P, 4], f32, tag="asf")
            nc.vector.tensor_tensor(out=asf[:], in0=selfr[:, HC:HC + 4],
                                    in1=selfr[:, HC + 4:HC + 8],
                                    op=mybir.AluOpType.add)
            nc.vector.tensor_tensor(out=asf[:], in0=asf[:], in1=lae[:],
                                    op=mybir.AluOpType.add)
            es = wpool.tile([P, 4], f32, tag="es")
            nc.scalar.activation(es[:], asf[:],
                                 mybir.ActivationFunctionType.Relu,
                                 scale=1.0 - NEG_SLOPE)
            nc.vector.scalar_tensor_tensor(
                out=es[:], in0=asf[:], scalar=NEG_SLOPE, in1=es[:],
                op0=mybir.AluOpType.mult, op1=mybir.AluOpType.add)
            nc.scalar.activation(es[:], es[:],
                                 mybir.ActivationFunctionType.Exp)
            # den = exp(alpha_self) + 1e-30 + sum_edges exp(alpha)
            den = wpool.tile([P, 4], f32, tag="den")
            nc.vector.scalar_tensor_tensor(
                out=den[:], in0=es[:], scalar=1e-30,
                in1=ps_agg[:, HC:HC + 4],
                op0=mybir.AluOpType.add, op1=mybir.AluOpType.add)
            rec = wpool.tile([P, 4], f32, tag="rec")
            nc.vector.reciprocal(rec[:], den[:])
            ot = opool.tile([P, HC], f32, tag="ot")
            es_b = es[:].unsqueeze(2).broadcast_to([P, 4, C_OUT])
            nc.vector.tensor_tensor(
                out=ot[:].rearrange("p (h c) -> p h c", c=C_OUT),
                in0=selfr[:, 0:HC].rearrange("p (h c) -> p h c", c=C_OUT),
                in1=es_b, op=mybir.AluOpType.mult)
            nc.vector.tensor_tensor(out=ot[:], in0=ot[:], in1=ps_agg[:, 0:HC],
                                    op=mybir.AluOpType.add)
            rec_b = rec[:].unsqueeze(2).broadcast_to([P, 4, C_OUT])
            nc.vector.tensor_tensor(
                out=ot[:].rearrange("p (h c) -> p h c", c=C_OUT),
                in0=ot[:].rearrange("p (h c) -> p h c", c=C_OUT),
                in1=rec_b, op=mybir.AluOpType.mult)
            nc.sync.dma_start(out[w * P:(w + 1) * P, :], ot[:])

    nc.compile()
    return nc


_NC_CACHE = {}


def _get_nc(cfg):
    k = cfg.key()
    if k not in _NC_CACHE:
        _NC_CACHE[k] = _build_nc(cfg)
    return _NC_CACHE[k]


def kernel(**inputs):
    x = np.asarray(inputs["x"], dtype=np.float32)
    ei = np.asarray(inputs["edge_index"])
    ea = np.asarray(inputs["edge_attr"], dtype=np.float32)
    W = np.asarray(inputs["W"], dtype=np.float32)
    W_edge = np.asarray(inputs["W_edge"], dtype=np.float32)
    att_src = np.asarray(inputs["att_src"], dtype=np.float32)
    att_dst = np.asarray(inputs["att_dst"], dtype=np.float32)
    att_edge = np.asarray(inputs["att_edge"], dtype=np.float32)
    bias = np.asarray(inputs["bias"], dtype=np.float32)

    src = ei[0].astype(np.int64)
    dst = ei[1].astype(np.int64)
    WallT, vT8 = _fold_weights(W, W_edge, att_src, att_dst, att_edge)

    cfg, in_maps, meta = _prep(x, src, dst, ea, WallT, vT8)
    nc = _get_nc(cfg)

    from concourse.bass_utils import run_bass_kernel_spmd
    res = run_bass_kernel_spmd(nc, in_maps, core_ids=list(range(NCORES)),
                               trace=TRACE)
    if TRACE:
        global LAST_RESULT
        LAST_RESULT = res

    out_ws = np.concatenate([res.results[c]["out"] for c in range(NCORES)],
                            axis=0)  # [NSLOTS, HC] in window space
    out = out_ws[meta["winpos"]]
    return (out + bias[None, :]).astype(np.float32)


# revision 7
# speedup vs baseline: 3.0371x; 1.1667x over previous
"""GAT message-passing kernel for Trainium2, 8 NeuronCores, dst-partitioned.

v2 (bf16 + batched one-hots + rotated SWDGE queues):
 - Fold attention vectors into the linear weights on host (tiny matmuls):
   a_src = x @ u_src.T, a_dst = x @ u_dst.T, a_edge = edge_attr @ v.T.
 - Softmax over incoming edges is computed WITHOUT max-subtraction (logits
   are bounded so exp cannot overflow; softmax is shift-invariant) so only
   segment-SUMS are needed, which map onto TensorE one-hot matmuls.
 - Host packs destination nodes into 128-slot windows (slot 127 of every
   window is reserved as a trash slot for padded edges), balanced by
   in-degree (LPT), orders/pads edges by window, ships per-core indices.
 - Everything on device runs in bf16 (tolerance 2e-2; bf16 keeps ~0.4%/elem)
   which makes PE matmuls 4x faster and halves gather bytes.
 - Node table [xh | a_src | a_dst] is bf16 256B rows in DRAM; per-edge rows
   are fetched with dma_gather (int16 idx, split low/high tables so both are
   int16-indexable). Padded slots have idx=-1 which the Q7 kernel trims from
   the tail; their dstloc points at the reserved slot 127 so stale SBUF data
   lands in a discarded output column (G buffers are zeroed once at start so
   stale data is never NaN).
 - Gathers rotate across the 4 SWDGE queues (4 Q7 core pairs + descriptor
   rings) which overlaps descriptor generation with DMA drain: measured
   ~3ns/row vs ~12ns/row on a single queue.
 - Per window, both one-hot matrices are built in ONE vector op each:
   S  [e, n] = (iota_col == dstloc_e)   (agg matmul lhsT, edge on partition)
   St [n, e] = (dstlocT == iota_part)   (a_dst expansion lhsT)
   dstlocT rows are broadcast to all partitions by a stride-0 DMA read.
 - Self-loops (PyG GATConv: loop edge_attr = per-dst mean of incoming
   edge_attr) fold in at window close from the unweighted aedge segment-sum.
"""

import math

import numpy as np

NCORES = 8
D_IN = 128
H_HEADS = 4
C_OUT = 16
HC = H_HEADS * C_OUT  # 64
ED_DIM = 64
NEG_SLOPE = 0.2
TW = 128             # table row width (bf16) -> 256B rows for dma_gather

P = 128  # partitions / window slot count (127 real nodes + trash slot)

TRACE = False       # set by test harness to capture an NTFF profile
LAST_RESULT = None  # BassKernelResults of the last traced run


class _Cfg:
    def __init__(self, nwl, kl, kh, nt_pad, nslots, split):
        self.NWL = nwl            # windows per core
        self.KL = kl              # low-half edge blocks per window
        self.KH = kh              # high-half edge blocks per window
        self.K = kl + kh          # 128-edge blocks per window
        self.NT_PAD = nt_pad      # node-table rows
        self.NSLOTS = nslots      # NCORES*NWL*128 window-space node slots
        self.SPLIT = split        # table row where the high half starts
        self.ECB = nwl * self.K   # edge blocks per core

    def key(self):
        return (self.NWL, self.KL, self.KH, self.NT_PAD, self.NSLOTS,
                self.SPLIT)


def _fold_weights(W, W_edge, att_src, att_dst, att_edge):
    H, C = att_src.shape
    D = W.shape[1]
    ED = W_edge.shape[1]
    u_src = np.einsum("hc,hcd->hd", att_src, W.reshape(H, C, D))
    u_dst = np.einsum("hc,hcd->hd", att_dst, W.reshape(H, C, D))
    v = np.einsum("hc,hcd->hd", att_edge, W_edge.reshape(H, C, ED))
    # WallT columns = [W.T | u_src.T | u_dst.T | zero pad to TW]
    WallT = np.zeros((D, TW), np.float32)
    WallT[:, :HC] = W.T
    WallT[:, HC:HC + H] = u_src.T
    WallT[:, HC + H:HC + 2 * H] = u_dst.T
    # vT8: rows 0:ED -> [v.T | 0], rows ED:2ED -> [0 | v.T]  (paired matmul)
    vT8 = np.zeros((2 * ED, 2 * H), np.float32)
    vT8[:ED, :H] = v.T
    vT8[ED:, H:] = v.T
    return WallT, vT8


def _partition_nodes(dst, n_nodes, n_windows):
    """LPT-pack nodes into n_windows bins of <=127 nodes each (slot 127 is
    reserved), balancing in-degree sums."""
    import heapq

    cap = P - 1
    deg = np.bincount(dst, minlength=n_nodes).astype(np.int64)
    order = np.argsort(-deg, kind="stable")
    heap = [(0, w) for w in range(n_windows)]
    heapq.heapify(heap)
    win_of = np.empty(n_nodes, np.int32)
    slot_of = np.empty(n_nodes, np.int32)
    nodes_in = np.zeros(n_windows, np.int32)
    edges_in = np.zeros(n_windows, np.int64)
    for n in order:
        while True:
            e, w = heapq.heappop(heap)
            if nodes_in[w] < cap:
                break  # full windows are dropped from the heap for good
        win_of[n] = w
        slot_of[n] = nodes_in[w]
        nodes_in[w] += 1
        edges_in[w] += deg[n]
        if nodes_in[w] < cap:
            heapq.heappush(heap, (int(edges_in[w]), w))
    return win_of, slot_of


def _wrap16(idx, num):
    """int16 index array -> dma_gather layout: item i lives at partition
    i%16, col i//16; replicated down the remaining 112 partitions."""
    a = idx.astype(np.int16).reshape(num // 16, 16).T  # [16, num//16]
    return np.ascontiguousarray(np.tile(a, (8, 1)))


def _prep(x, src, dst, edge_attr, WallT, vT8):
    """Build per-core input maps + meta for unsharding."""
    import ml_dtypes
    bf = ml_dtypes.bfloat16

    n = x.shape[0]
    nwl = math.ceil(n / ((P - 1) * NCORES))
    n_windows = NCORES * nwl
    nslots = n_windows * P
    nt_pad = nslots
    split = (nslots // 2 // P) * P
    assert split <= 32767 and nt_pad - split <= 32767

    win_of, slot_of = _partition_nodes(dst, n, n_windows)
    R_TRASH = P - 1

    winpos = win_of.astype(np.int64) * P + slot_of

    ewin = win_of[dst]
    srow = winpos[src]
    is_low = srow < split

    # fixed per-window low/high block counts across all cores (SPMD)
    nlow = np.bincount(ewin[is_low], minlength=n_windows)
    nhigh = np.bincount(ewin[~is_low], minlength=n_windows)
    kl = max(1, math.ceil(nlow.max() / P))
    kh = max(1, math.ceil(nhigh.max() / P))
    if (kl + kh) % 2:
        kh += 1  # keep total block count even for paired phase-E matmuls
    cfg = _Cfg(nwl, kl, kh, nt_pad, nslots, split)
    K = cfg.K
    epw = K * P

    # ---- place edges: window-major [low | pad(-1) | high | pad(-1)] ----
    grp = ewin.astype(np.int64) * 2 + (~is_low)
    order_e = np.argsort(grp, kind="stable")
    grp_s = grp[order_e]
    counts = np.bincount(grp_s, minlength=2 * n_windows)
    offs = np.zeros(2 * n_windows + 1, np.int64)
    np.cumsum(counts, out=offs[1:])
    pos = np.arange(len(order_e), dtype=np.int64) - offs[grp_s]
    q = (grp_s // 2) * epw + (grp_s % 2) * (kl * P) + pos

    Q = n_windows * epw
    lowmask_q = (np.arange(Q) % epw) < kl * P
    gsrc_q = np.full(Q, -1, np.int64)
    dstloc_q = np.full(Q, R_TRASH, np.uint8)
    gsrc_q[q] = srow[order_e]
    dstloc_q[q] = slot_of[dst[order_e]].astype(np.uint8)
    dstloc_q = dstloc_q.astype(bf)

    ea_q = np.zeros((Q, ED_DIM), np.float32)
    ea_q[q] = edge_attr[order_e]

    # window-space node features (zero for empty slots)
    x_ws = np.zeros((nslots, D_IN), np.float32)
    x_ws[winpos] = x
    xT = np.zeros((D_IN, nt_pad), bf)
    xT[:, :nslots] = x_ws.T.astype(bf)

    invcnt_ws = np.ones(nslots, np.float32)
    cnt = np.bincount(dst, minlength=n).astype(np.float32)
    invcnt_ws[winpos] = 1.0 / np.maximum(cnt, 1.0)

    PAD_IDX = 0  # -1 enables Q7 tail-trimming; 0 gathers row 0 (debug)
    glow_q = np.where(lowmask_q & (gsrc_q >= 0), gsrc_q, PAD_IDX)
    ghigh_q = np.where(~lowmask_q & (gsrc_q >= 0), gsrc_q - split, PAD_IDX)
    assert glow_q.max() < split and ghigh_q.max() < nt_pad - split

    in_maps = []
    pcr = nwl * P
    WallT16 = WallT.astype(bf)
    vT816 = vT8.astype(bf)
    for c in range(NCORES):
        qs, qe = c * nwl * epw, (c + 1) * nwl * epw
        eac = ea_q[qs:qe].reshape(nwl * K // 2, 2, P, ED_DIM)
        eaT2 = np.ascontiguousarray(
            eac.transpose(1, 3, 0, 2).reshape(2 * ED_DIM, -1)).astype(bf)
        dstloc_c = np.ascontiguousarray(
            dstloc_q[qs:qe].reshape(nwl * K, P).T)  # [P, ECB] u8
        dstlocT_c = np.ascontiguousarray(
            dstloc_q[qs:qe].reshape(nwl, epw))      # [NWL, K*128] u8
        lo = glow_q[qs:qe].reshape(nwl, epw)
        hi = ghigh_q[qs:qe].reshape(nwl, epw)
        glo16 = np.concatenate(
            [_wrap16(lo[w, :kl * P], kl * P) for w in range(nwl)], axis=1)
        ghi16 = np.concatenate(
            [_wrap16(hi[w, kl * P:], kh * P) for w in range(nwl)], axis=1)
        invcnt_c = np.ascontiguousarray(
            invcnt_ws[c * pcr:(c + 1) * pcr].reshape(nwl, P).T
            .astype(np.float32))
        in_maps.append(dict(
            xT=xT, eaT2=eaT2, dstloc=dstloc_c, dstlocT=dstlocT_c,
            invcnt=invcnt_c, glo16=glo16, ghi16=ghi16,
            WallT=WallT16, vT8=vT816,
        ))
    meta = dict(winpos=winpos, cfg=cfg)
    return cfg, in_maps, meta


def _build_nc(cfg):
    import concourse.bass as bass
    import concourse.tile as tile
    from concourse import bacc, mybir
    from contextlib import ExitStack

    f32 = mybir.dt.float32
    bf16 = mybir.dt.bfloat16
    i16 = mybir.dt.int16
    u8 = mybir.dt.uint8
    NWL, KL, KH, K = cfg.NWL, cfg.KL, cfg.KH, cfg.K
    NT_PAD, SPLIT = cfg.NT_PAD, cfg.SPLIT
    ECB = cfg.ECB
    PCR = NWL * P
    UH = H_HEADS

    nc = bacc.Bacc("TRN2", target_bir_lowering=False, debug=False,
                   num_devices=NCORES, num_swdge_queues=4,
                   dynamic_dma_scratch_size=65536)
    xT = nc.dram_tensor("xT", [D_IN, NT_PAD], bf16, kind="ExternalInput").ap()
    WallT = nc.dram_tensor("WallT", [D_IN, TW], bf16,
                           kind="ExternalInput").ap()
    vT8 = nc.dram_tensor("vT8", [2 * ED_DIM, 2 * H_HEADS], bf16,
                         kind="ExternalInput").ap()
    eaT2 = nc.dram_tensor("eaT2", [2 * ED_DIM, ECB * P // 2], bf16,
                          kind="ExternalInput").ap()
    dstloc = nc.dram_tensor("dstloc", [P, ECB], bf16,
                            kind="ExternalInput").ap()
    dstlocT = nc.dram_tensor("dstlocT", [NWL, K * P], bf16,
                             kind="ExternalInput").ap()
    invcnt = nc.dram_tensor("invcnt", [P, NWL], f32, kind="ExternalInput").ap()
    glo16 = nc.dram_tensor("glo16", [P, NWL * KL * 8], i16,
                           kind="ExternalInput").ap()
    ghi16 = nc.dram_tensor("ghi16", [P, NWL * KH * 8], i16,
                           kind="ExternalInput").ap()
    out = nc.dram_tensor("out", [PCR, HC], f32, kind="ExternalOutput").ap()
    tableA = nc.dram_tensor("tableA", [NT_PAD, TW], bf16).ap()

    with tile.TileContext(nc) as tc, ExitStack() as ctx:
        cpool = ctx.enter_context(tc.tile_pool(name="const", bufs=1))
        xpool = ctx.enter_context(tc.tile_pool(name="xload", bufs=3))
        tabpool = ctx.enter_context(tc.tile_pool(name="tab", bufs=3))
        eapool = ctx.enter_context(tc.tile_pool(name="ea", bufs=2))
        gpool = ctx.enter_context(tc.tile_pool(name="gather", bufs=4))
        dtpool = ctx.enter_context(tc.tile_pool(name="dstT", bufs=2))
        onepool = ctx.enter_context(tc.tile_pool(name="onehot", bufs=2))
        mpool = ctx.enter_context(tc.tile_pool(name="msg", bufs=2))
        wpool = ctx.enter_context(tc.tile_pool(name="work", bufs=3))
        opool = ctx.enter_context(tc.tile_pool(name="outw", bufs=3))
        pst = ctx.enter_context(tc.tile_pool(name="ps_t", bufs=2, space="PSUM"))
        pse = ctx.enter_context(tc.tile_pool(name="ps_e", bufs=2, space="PSUM"))
        psa = ctx.enter_context(tc.tile_pool(name="ps_a", bufs=2, space="PSUM"))
        psad = ctx.enter_context(tc.tile_pool(name="ps_ad", bufs=2,
                                              space="PSUM"))

        # ---- constants ----
        WallT_sb = cpool.tile([P, TW], bf16)
        nc.sync.dma_start(WallT_sb[:], WallT[:])
        vT8_sb = cpool.tile([2 * ED_DIM, 2 * H_HEADS], bf16)
        nc.sync.dma_start(vT8_sb[:], vT8[:])
        iota_rep = cpool.tile([P, K * P], bf16)  # value = col % 128
        nc.gpsimd.iota(iota_rep[:].rearrange("p (k u) -> p k u", u=P),
                       pattern=[[0, K], [1, P]], base=0,
                       channel_multiplier=0,
                       allow_small_or_imprecise_dtypes=True)
        piota = cpool.tile([P, 1], f32)  # value = partition idx
        nc.gpsimd.iota(piota[:], pattern=[[0, 1]], base=0,
                       channel_multiplier=1,
                       allow_small_or_imprecise_dtypes=True)
        glo_sb = cpool.tile([P, NWL * KL * 8], i16)
        nc.sync.dma_start(glo_sb[:], glo16[:])
        ghi_sb = cpool.tile([P, NWL * KH * 8], i16)
        nc.sync.dma_start(ghi_sb[:], ghi16[:])
        dstloc_sb = cpool.tile([P, ECB], bf16)
        nc.sync.dma_start(dstloc_sb[:], dstloc[:])
        invcnt_sb = cpool.tile([P, NWL], f32)
        nc.sync.dma_start(invcnt_sb[:], invcnt[:])

        # zero the gather buffers once: trimmed (padded) slots keep stale
        # data, which must be finite (never NaN from a previous NEFF)
        for _ in range(4):
            G0 = gpool.tile([P, K * TW], bf16, tag="G")
            nc.vector.memset(G0[:], 0.0)

        # ---- phase T: node table = [xh | a_src | a_dst | 0 pad] ----
        NTT = NT_PAD // P
        XB = 8
        for g in range(math.ceil(NTT / XB)):
            t0 = g * XB
            nt = min(XB, NTT - t0)
            xt = xpool.tile([P, XB * P], bf16, tag="xt")
            nc.sync.dma_start(xt[:, :nt * P], xT[:, t0 * P:(t0 + nt) * P])
            tab = tabpool.tile([P, XB * TW], bf16, tag="tab")
            for t4 in range(0, nt, 4):
                n4 = min(4, nt - t4)
                ps = pst.tile([P, 4 * TW], f32)
                for t in range(t4, t4 + n4):
                    nc.tensor.matmul(out=ps[:, (t - t4) * TW:(t - t4 + 1) * TW],
                                     lhsT=xt[:, t * P:(t + 1) * P],
                                     rhs=WallT_sb[:], start=True, stop=True)
                if (t4 // 4) % 2 == 0:
                    nc.vector.tensor_copy(
                        tab[:, t4 * TW:(t4 + n4) * TW], ps[:, :n4 * TW])
                else:
                    nc.scalar.copy(
                        tab[:, t4 * TW:(t4 + n4) * TW], ps[:, :n4 * TW])
            nc.scalar.dma_start(
                out=tableA[t0 * P:(t0 + nt) * P, :]
                .rearrange("(t p) u -> p t u", p=P),
                in_=tab[:, :nt * TW].rearrange("p (t u) -> p t u", u=TW))

        # ---- own node rows, straight into SBUF (partition-id offset) ----
        selfall = cpool.tile([P, NWL * (HC + 8)], bf16)
        base = nc.sync.partition_id() * PCR
        nc.sync.dma_start(
            out=selfall[:].rearrange("p (w u) -> p w u", u=HC + 8),
            in_=tableA[bass.ds(base, PCR), 0:HC + 8]
            .rearrange("(w p) u -> p w u", p=P))

        # ---- phase B: per-window attention softmax + aggregation ----
        for w in range(NWL):
            G = gpool.tile([P, K * TW], bf16, tag="G")
            Gv = G[:].rearrange("p (k u) -> p k u", u=TW)
            nc.gpsimd.dma_gather(
                out_ap=Gv[:, 0:KL, :], in_ap=tableA[0:SPLIT, :],
                idxs_ap=glo_sb[:, w * KL * 8:(w + 1) * KL * 8],
                num_idxs=KL * P, num_idxs_reg=KL * P, elem_size=TW,
                single_packet=False, queue_num=(2 * w) % 4)
            nc.gpsimd.dma_gather(
                out_ap=Gv[:, KL:K, :], in_ap=tableA[SPLIT:NT_PAD, :],
                idxs_ap=ghi_sb[:, w * KH * 8:(w + 1) * KH * 8],
                num_idxs=KH * P, num_idxs_reg=KH * P, elem_size=TW,
                single_packet=False, queue_num=(2 * w + 1) % 4)

            selfr = selfall[:, w * (HC + 8):(w + 1) * (HC + 8)]

            # one-hot builds (one vector op each, all K blocks at once)
            dT = dtpool.tile([P, K * P], bf16, tag="dT")
            nc.sync.dma_start(dT[:], dstlocT[w:w + 1, :]
                              .broadcast_to([P, K * P]))
            St = onepool.tile([P, K * P], bf16, tag="St")
            nc.vector.tensor_scalar(
                out=St[:], in0=dT[:], scalar1=piota[:], scalar2=None,
                op0=mybir.AluOpType.is_equal)
            S = onepool.tile([P, K * P], bf16, tag="S")
            nc.vector.tensor_tensor(
                out=S[:].rearrange("p (k u) -> p k u", u=P),
                in0=iota_rep[:].rearrange("p (k u) -> p k u", u=P),
                in1=dstloc_sb[:, w * K:(w + 1) * K].unsqueeze(2)
                .broadcast_to([P, K, P]),
                op=mybir.AluOpType.is_equal)

            # a_dst(dst) per edge: St-block matmuls against own a_dst rows
            ps_adst = psad.tile([P, K * UH], f32)
            for j in range(K):
                nc.tensor.matmul(out=ps_adst[:, j * UH:(j + 1) * UH],
                                 lhsT=St[:, j * P:(j + 1) * P],
                                 rhs=selfr[:, HC + 4:HC + 8],
                                 start=True, stop=True)

            # phase E slice for this window: a_edge0 = edge_attr @ v.T
            ea_ch = eapool.tile([2 * ED_DIM, (K // 2) * P], bf16, tag="ea")
            nc.sync.dma_start(
                ea_ch[:], eaT2[:, w * (K // 2) * P:(w + 1) * (K // 2) * P])
            ps_e = pse.tile([P, K * UH], f32)
            for jj in range(K // 2):
                nc.tensor.matmul(
                    out=ps_e[:, jj * 8:(jj + 1) * 8],
                    lhsT=ea_ch[:, jj * P:(jj + 1) * P],
                    rhs=vT8_sb[:], start=True, stop=True)

            # alpha = a_src(src) + a_dst(dst) + a_edge
            aw = wpool.tile([P, K * UH], f32, tag="aw")
            aw3 = aw[:].rearrange("p (k u) -> p k u", u=UH)
            nc.vector.tensor_tensor(
                out=aw3, in0=Gv[:, :, HC:HC + UH],
                in1=ps_adst[:].rearrange("p (k u) -> p k u", u=UH),
                op=mybir.AluOpType.add)
            nc.vector.tensor_tensor(
                out=aw[:], in0=aw[:], in1=ps_e[:], op=mybir.AluOpType.add)
            # lrelu(x) = slope*x + relu((1-slope)*x), then exp
            lrl = wpool.tile([P, K * UH], f32, tag="lrl")
            nc.scalar.activation(lrl[:], aw[:],
                                 mybir.ActivationFunctionType.Relu,
                                 scale=1.0 - NEG_SLOPE)
            nc.vector.scalar_tensor_tensor(
                out=lrl[:], in0=aw[:], scalar=NEG_SLOPE, in1=lrl[:],
                op0=mybir.AluOpType.mult, op1=mybir.AluOpType.add)

            # M = [expal * xh | expal | a_edge0] per block
            M = mpool.tile([P, K * (HC + 8)], bf16, tag="M")
            M3 = M[:].rearrange("p (k u) -> p k u", u=HC + 8)
            nc.scalar.activation(M3[:, :, HC:HC + UH],
                                 lrl[:].rearrange("p (k u) -> p k u", u=UH),
                                 mybir.ActivationFunctionType.Exp)
            nc.vector.tensor_copy(
                M3[:, :, HC + 4:HC + 8],
                ps_e[:].rearrange("p (k u) -> p k u", u=UH))
            expal_b = (M3[:, :, HC:HC + UH].unsqueeze(3)
                       .broadcast_to([P, K, UH, C_OUT]))
            nc.vector.tensor_tensor(
                out=M3[:, :, 0:HC].rearrange("p k (h c) -> p k h c", c=C_OUT),
                in0=Gv[:, :, 0:HC].rearrange("p k (h c) -> p k h c", c=C_OUT),
                in1=expal_b, op=mybir.AluOpType.mult)

            # segment sums: one matmul per block, accumulated in PSUM
            ps_agg = psa.tile([P, HC + 8], f32)
            for j in range(K):
                nc.tensor.matmul(out=ps_agg[:], lhsT=S[:, j * P:(j + 1) * P],
                                 rhs=M[:, j * (HC + 8):(j + 1) * (HC + 8)],
                                 start=(j == 0), stop=(j == K - 1))

            # ---- window close: self-loop term + normalization ----
            lae = wpool.tile([P, 4], f32, tag="lae")
            nc.vector.tensor_scalar(out=lae[:], in0=ps_agg[:, HC + 4:HC + 8],
                                    scalar1=invcnt_sb[:, w:w + 1],
                                    scalar2=None, op0=mybir.AluOpType.mult)
            asf = wpool.tile([P, 4], f32, tag="asf")
            nc.vector.tensor_tensor(out=asf[:], in0=selfr[:, HC:HC + 4],
                                    in1=selfr[:, HC + 4:HC + 8],
                                    op=mybir.AluOpType.add)
            nc.vector.tensor_tensor(out=asf[:], in0=asf[:], in1=lae[:],
                                    op=mybir.AluOpType.add)
            es = wpool.tile([P, 4], f32, tag="es")
            nc.scalar.activation(es[:], asf[:],
                                 mybir.ActivationFunctionType.Relu,
                                 scale=1.0 - NEG_SLOPE)
            nc.vector.scalar_tensor_tensor(
                out=es[:], in0=asf[:], scalar=NEG_SLOPE, in1=es[:],
                op0=mybir.AluOpType.mult, op1=mybir.AluOpType.add)
            nc.scalar.activation(es[:], es[:],
                                 mybir.ActivationFunctionType.Exp)
            # den = exp(alpha_self) + 1e-30 + sum_edges exp(alpha)
            den = wpool.tile([P, 4], f32, tag="den")
            nc.vector.scalar_tensor_tensor(
                out=den[:], in0=es[:], scalar=1e-30,
                in1=ps_agg[:, HC:HC + 4],
                op0=mybir.AluOpType.add, op1=mybir.AluOpType.add)
            rec = wpool.tile([P, 4], f32, tag="rec")
            nc.vector.reciprocal(rec[:], den[:])
            ot = opool.tile([P, HC], f32, tag="ot")
            es_b = es[:].unsqueeze(2).broadcast_to([P, 4, C_OUT])
            nc.vector.tensor_tensor(
                out=ot[:].rearrange("p (h c) -> p h c", c=C_OUT),
                in0=selfr[:, 0:HC].rearrange("p (h c) -> p h c", c=C_OUT),
                in1=es_b, op=mybir.AluOpType.mult)
            nc.vector.tensor_tensor(out=ot[:], in0=ot[:], in1=ps_agg[:, 0:HC],
                                    op=mybir.AluOpType.add)
            rec_b = rec[:].unsqueeze(2).broadcast_to([P, 4, C_OUT])
            nc.vector.tensor_tensor(
                out=ot[:].rearrange("p (h c) -> p h c", c=C_OUT),
                in0=ot[:].rearrange("p (h c) -> p h c", c=C_OUT),
                in1=rec_b, op=mybir.AluOpType.mult)
            nc.sync.dma_start(out[w * P:(w + 1) * P, :], ot[:])

    nc.compile()
    return nc


_NC_CACHE = {}


def _get_nc(cfg):
    k = cfg.key()
    if k not in _NC_CACHE:
        _NC_CACHE[k] = _build_nc(cfg)
    return _NC_CACHE[k]


def kernel(**inputs):
    x = np.asarray(inputs["x"], dtype=np.float32)
    ei = np.asarray(inputs["edge_index"])
    ea = np.asarray(inputs["edge_attr"], dtype=np.float32)
    W = np.asarray(inputs["W"], dtype=np.float32)
    W_edge = np.asarray(inputs["W_edge"], dtype=np.float32)
    att_src = np.asarray(inputs["att_src"], dtype=np.float32)
    att_dst = np.asarray(inputs["att_dst"], dtype=np.float32)
    att_edge = np.asarray(inputs["att_edge"], dtype=np.float32)
    bias = np.asarray(inputs["bias"], dtype=np.float32)

    src = ei[0].astype(np.int64)
    dst = ei[1].astype(np.int64)
    WallT, vT8 = _fold_weights(W, W_edge, att_src, att_dst, att_edge)

    cfg, in_maps, meta = _prep(x, src, dst, ea, WallT, vT8)
    nc = _get_nc(cfg)

    from concourse.bass_utils import run_bass_kernel_spmd
    res = run_bass_kernel_spmd(nc, in_maps, core_ids=list(range(NCORES)),
                               trace=TRACE)
    if TRACE:
        global LAST_RESULT
        LAST_RESULT = res

    out_ws = np.concatenate([res.results[c]["out"] for c in range(NCORES)],
                            axis=0)  # [NSLOTS, HC] in window space
    out = out_ws[meta["winpos"]]
    return (out + bias[None, :]).astype(np.float32)


# revision 9
# speedup vs baseline: 4.1994x; 1.3827x over previous
"""GAT message-passing kernel for Trainium2, 8 NeuronCores, dst-partitioned.

v3 (bf16, paired windows, shipped transposed one-hot, p-major table):
 - Fold attention vectors into the linear weights on host (tiny matmuls):
   a_src = x @ u_src.T, a_dst = x @ u_dst.T, a_edge = edge_attr @ v.T.
 - Softmax over incoming edges is computed WITHOUT max-subtraction (logits
   are bounded so exp cannot overflow; softmax is shift-invariant) so only
   segment-SUMS are needed, which map onto TensorE one-hot matmuls.
 - Host packs destination nodes into 128-slot windows (slot 127 of every
   window is a trash slot that absorbs padded edges), balanced by in-degree
   (LPT). Windows are processed in PAIRS: one gather instruction per table
   half per pair (~2300 rows each) amortizes the ~1us SWDGE fixed cost;
   gathers rotate across the 4 SWDGE queues (4 Q7 core pairs + descriptor
   rings), which overlaps descriptor generation with DMA drain.
 - Everything on device is bf16 (tolerance 2e-2; bf16 adds ~0.5%):
   PE matmuls run 4x faster and gather rows are 256B.
 - Node table rows are PARTITION-MAJOR: row(node) = slot*NTT + window, so
   phase T writes the table with one contiguous 2KB descriptor per
   partition per chunk instead of 256B/row descriptors, and a core's own
   rows [slot, ds(core*NWL+w)] load straight into SBUF with one DMA.
   Low/high table split (int16 gather indices) is slot<64 vs slot>=64.
 - Per pair, the one-hot S [edge, node] is built in ONE DVE op (iota vs
   dstloc broadcast); the transposed one-hot St [node, edge] (lhsT of the
   a_dst expansion matmul) is SHIPPED from host — it is pure index data,
   and DVE compare/broadcast ops run at ~1/4 copy speed, so a DMA is
   cheaper than rebuilding or PE-transposing (which needs a PSUM round
   trip of the same size).
 - Self-loops (PyG GATConv: loop edge_attr = per-dst mean of incoming
   edge_attr) fold in at window close from the unweighted aedge segment
   sum that rides the aggregation matmul.
"""

import math

import numpy as np

NCORES = 8
D_IN = 128
H_HEADS = 4
C_OUT = 16
HC = H_HEADS * C_OUT  # 64
ED_DIM = 64
NEG_SLOPE = 0.2
TW = 128             # table row width (bf16) -> 256B rows for dma_gather
UH = H_HEADS
MW = HC + 8          # M columns per block: [expal*xh | expal | a_edge0]

P = 128  # partitions / window slot count (127 real nodes + trash slot)

TRACE = False       # set by test harness to capture an NTFF profile
LAST_RESULT = None  # BassKernelResults of the last traced run


class _Cfg:
    def __init__(self, nwl, kl, kh, ncores):
        self.NWL = nwl            # windows per core (even)
        self.NPAIR = nwl // 2
        self.KL = kl              # low-half edge blocks per window
        self.KH = kh              # high-half edge blocks per window
        self.K = kl + kh          # 128-edge blocks per window
        self.K2 = 2 * self.K      # blocks per window pair
        self.EPP = self.K2 * P    # edge slots per window pair
        self.NTT = ncores * nwl   # table rows per slot (p-major layout)
        self.NT_PAD = self.NTT * P
        self.NSLOTS = self.NT_PAD
        self.SPLIT = 64 * self.NTT  # rows with slot<64
        self.ECB = nwl * self.K   # edge blocks per core

    def key(self):
        return (self.NWL, self.KL, self.KH, self.NTT)


def _fold_weights(W, W_edge, att_src, att_dst, att_edge):
    H, C = att_src.shape
    D = W.shape[1]
    ED = W_edge.shape[1]
    u_src = np.einsum("hc,hcd->hd", att_src, W.reshape(H, C, D))
    u_dst = np.einsum("hc,hcd->hd", att_dst, W.reshape(H, C, D))
    v = np.einsum("hc,hcd->hd", att_edge, W_edge.reshape(H, C, ED))
    # WallT columns = [W.T | u_src.T | u_dst.T | zero pad to TW]
    WallT = np.zeros((D, TW), np.float32)
    WallT[:, :HC] = W.T
    WallT[:, HC:HC + H] = u_src.T
    WallT[:, HC + H:HC + 2 * H] = u_dst.T
    # vT8: rows 0:ED -> [v.T | 0], rows ED:2ED -> [0 | v.T]  (paired matmul)
    vT8 = np.zeros((2 * ED, 2 * H), np.float32)
    vT8[:ED, :H] = v.T
    vT8[ED:, H:] = v.T
    return WallT, vT8


def _partition_nodes(dst, n_nodes, n_windows):
    """LPT-pack nodes into n_windows bins of <=127 nodes each (slot 127 is
    the trash slot), balancing in-degree sums."""
    import heapq

    cap = P - 1
    deg = np.bincount(dst, minlength=n_nodes).astype(np.int64)
    order = np.argsort(-deg, kind="stable")
    heap = [(0, w) for w in range(n_windows)]
    heapq.heapify(heap)
    win_of = np.empty(n_nodes, np.int32)
    slot_of = np.empty(n_nodes, np.int32)
    nodes_in = np.zeros(n_windows, np.int32)
    edges_in = np.zeros(n_windows, np.int64)
    for n in order:
        while True:
            e, w = heapq.heappop(heap)
            if nodes_in[w] < cap:
                break  # full windows are dropped from the heap for good
        win_of[n] = w
        slot_of[n] = nodes_in[w]
        nodes_in[w] += 1
        edges_in[w] += deg[n]
        if nodes_in[w] < cap:
            heapq.heappush(heap, (int(edges_in[w]), w))
    return win_of, slot_of


def _wrap16(idx, num):
    """int16 index array -> dma_gather layout: item i lives at partition
    i%16, col i//16; replicated down the remaining 112 partitions."""
    a = idx.astype(np.int16).reshape(num // 16, 16).T  # [16, num//16]
    return np.ascontiguousarray(np.tile(a, (8, 1)))


def _prep(x, src, dst, edge_attr, WallT, vT8):
    """Build per-core input maps + meta for unsharding."""
    import ml_dtypes
    bf = ml_dtypes.bfloat16

    n = x.shape[0]
    nwl = math.ceil(n / ((P - 1) * NCORES))
    if nwl % 2:
        nwl += 1  # windows are processed in pairs
    n_windows = NCORES * nwl

    win_of, slot_of = _partition_nodes(dst, n, n_windows)
    R_TRASH = P - 1

    winpos = win_of.astype(np.int64) * P + slot_of
    ntt = n_windows
    # p-major table row: row = slot * NTT + global_window
    row_of = slot_of.astype(np.int64) * ntt + win_of
    split = 64 * ntt
    assert split <= 32768 and ntt * P - split <= 32767

    ewin = win_of[dst]
    srow = row_of[src]
    is_low = (slot_of[src] < 64)

    # fixed per-window low/high block counts across all cores (SPMD)
    nlow = np.bincount(ewin[is_low], minlength=n_windows)
    nhigh = np.bincount(ewin[~is_low], minlength=n_windows)
    kl = max(1, math.ceil(nlow.max() / P))
    kh = max(1, math.ceil(nhigh.max() / P))
    cfg = _Cfg(nwl, kl, kh, NCORES)
    K2, EPP = cfg.K2, cfg.EPP
    npair_g = n_windows // 2

    # ---- place edges pair-major: [low(2v) | low(2v+1) | hi(2v) | hi(2v+1)],
    #      each region padded to a block multiple ----
    pairg = ewin.astype(np.int64) // 2
    parity = ewin.astype(np.int64) % 2
    half = (~is_low).astype(np.int64)
    grp = pairg * 4 + half * 2 + parity
    order_e = np.argsort(grp, kind="stable")
    grp_s = grp[order_e]
    counts = np.bincount(grp_s, minlength=4 * npair_g)
    offs = np.zeros(4 * npair_g + 1, np.int64)
    np.cumsum(counts, out=offs[1:])
    pos = np.arange(len(order_e), dtype=np.int64) - offs[grp_s]
    roff = np.array([0, kl * P, 2 * kl * P, (2 * kl + kh) * P], np.int64)
    q = (grp_s // 4) * EPP + roff[grp_s % 4] + pos

    Q = npair_g * EPP
    lowmask_q = (np.arange(Q) % EPP) < 2 * kl * P
    gsrc_q = np.zeros(Q, np.int64)  # pads gather row 0 (harmless: trash dst)
    dstloc_q = np.full(Q, R_TRASH, np.int16)
    gsrc_q[q] = srow[order_e]
    dstloc_q[q] = slot_of[dst[order_e]].astype(np.int16)

    ea_q = np.zeros((Q, ED_DIM), np.float32)
    ea_q[q] = edge_attr[order_e]

    # node features, columns in phase-T consumption order: chunk block t,
    # partition p -> table row p*NTT + t, so col t*128+p holds row p*ntt+t
    x_rows = np.zeros((cfg.NT_PAD, D_IN), np.float32)
    x_rows[row_of] = x
    xT = np.ascontiguousarray(
        x_rows.reshape(P, ntt, D_IN).transpose(1, 0, 2)
        .reshape(cfg.NT_PAD, D_IN).T.astype(bf))  # [D_IN, NT_PAD]

    invcnt_ws = np.ones(n_windows * P, np.float32)
    cnt = np.bincount(dst, minlength=n).astype(np.float32)
    invcnt_ws[winpos] = 1.0 / np.maximum(cnt, 1.0)

    glow_q = np.where(lowmask_q, gsrc_q, 0)
    ghigh_q = np.where(lowmask_q, 0, np.maximum(gsrc_q - split, 0))
    assert glow_q.max() < split and ghigh_q.max() < ntt * P - split

    in_maps = []
    npair = cfg.NPAIR
    WallT16 = WallT.astype(bf)
    vT816 = vT8.astype(bf)
    slot_ar = np.arange(P, dtype=np.int16)
    for c in range(NCORES):
        qs, qe = c * npair * EPP, (c + 1) * npair * EPP
        dq = dstloc_q[qs:qe]
        eac = ea_q[qs:qe].reshape(npair * K2 // 2, 2, P, ED_DIM)
        eaT2 = np.ascontiguousarray(
            eac.transpose(1, 3, 0, 2).reshape(2 * ED_DIM, -1)).astype(bf)
        dstloc_c = np.ascontiguousarray(
            dq.reshape(npair * K2, P).T.astype(bf))   # [P, NPAIR*K2]
        StA = np.ascontiguousarray(
            (dq[None, :] == slot_ar[:, None]).astype(bf))  # [P, NPAIR*EPP]
        lo = glow_q[qs:qe].reshape(npair, EPP)
        hi = ghigh_q[qs:qe].reshape(npair, EPP)
        glo16 = np.concatenate(
            [_wrap16(lo[v, :2 * kl * P], 2 * kl * P) for v in range(npair)],
            axis=1)
        ghi16 = np.concatenate(
            [_wrap16(hi[v, 2 * kl * P:], 2 * kh * P) for v in range(npair)],
            axis=1)
        invcnt_c = np.ascontiguousarray(
            invcnt_ws[c * nwl * P:(c + 1) * nwl * P].reshape(nwl, P).T
            .astype(np.float32))
        in_maps.append(dict(
            xT=xT, eaT2=eaT2, dstloc=dstloc_c, StA=StA,
            invcnt=invcnt_c, glo16=glo16, ghi16=ghi16,
            WallT=WallT16, vT8=vT816,
        ))
    meta = dict(winpos=winpos, cfg=cfg)
    return cfg, in_maps, meta


def _build_nc(cfg):
    import concourse.bass as bass
    import concourse.tile as tile
    from concourse import bacc, mybir
    from contextlib import ExitStack

    f32 = mybir.dt.float32
    bf16 = mybir.dt.bfloat16
    i16 = mybir.dt.int16
    NWL, NPAIR, KL, KH = cfg.NWL, cfg.NPAIR, cfg.KL, cfg.KH
    K, K2, EPP = cfg.K, cfg.K2, cfg.EPP
    NTT, NT_PAD, SPLIT = cfg.NTT, cfg.NT_PAD, cfg.SPLIT

    nc = bacc.Bacc("TRN2", target_bir_lowering=False, debug=False,
                   num_devices=NCORES, num_swdge_queues=4,
                   dynamic_dma_scratch_size=65536)
    xT = nc.dram_tensor("xT", [D_IN, NT_PAD], bf16, kind="ExternalInput").ap()
    WallT = nc.dram_tensor("WallT", [D_IN, TW], bf16,
                           kind="ExternalInput").ap()
    vT8 = nc.dram_tensor("vT8", [2 * ED_DIM, 2 * H_HEADS], bf16,
                         kind="ExternalInput").ap()
    eaT2 = nc.dram_tensor("eaT2", [2 * ED_DIM, NPAIR * K2 * P // 2], bf16,
                          kind="ExternalInput").ap()
    dstloc = nc.dram_tensor("dstloc", [P, NPAIR * K2], bf16,
                            kind="ExternalInput").ap()
    StA = nc.dram_tensor("StA", [P, NPAIR * EPP], bf16,
                         kind="ExternalInput").ap()
    invcnt = nc.dram_tensor("invcnt", [P, NWL], f32, kind="ExternalInput").ap()
    glo16 = nc.dram_tensor("glo16", [P, NPAIR * 2 * KL * 8], i16,
                           kind="ExternalInput").ap()
    ghi16 = nc.dram_tensor("ghi16", [P, NPAIR * 2 * KH * 8], i16,
                           kind="ExternalInput").ap()
    out = nc.dram_tensor("out", [NWL * P, HC], f32, kind="ExternalOutput").ap()
    tableA = nc.dram_tensor("tableA", [NT_PAD, TW], bf16).ap()
    tab3 = tableA.rearrange("(p t) u -> p t u", t=NTT)

    with tile.TileContext(nc) as tc, ExitStack() as ctx:
        cpool = ctx.enter_context(tc.tile_pool(name="const", bufs=1))
        xpool = ctx.enter_context(tc.tile_pool(name="xload", bufs=3))
        tabpool = ctx.enter_context(tc.tile_pool(name="tab", bufs=3))
        eapool = ctx.enter_context(tc.tile_pool(name="ea", bufs=2))
        gpool = ctx.enter_context(tc.tile_pool(name="gather", bufs=3))
        stpool = ctx.enter_context(tc.tile_pool(name="sT", bufs=2))
        spool = ctx.enter_context(tc.tile_pool(name="oneh", bufs=2))
        mpool = ctx.enter_context(tc.tile_pool(name="msg", bufs=2))
        wpool = ctx.enter_context(tc.tile_pool(name="work", bufs=3))
        opool = ctx.enter_context(tc.tile_pool(name="outw", bufs=3))
        pst = ctx.enter_context(tc.tile_pool(name="ps_t", bufs=2, space="PSUM"))
        pse = ctx.enter_context(tc.tile_pool(name="ps_e", bufs=2, space="PSUM"))
        psa = ctx.enter_context(tc.tile_pool(name="ps_a", bufs=2, space="PSUM"))
        psad = ctx.enter_context(tc.tile_pool(name="ps_ad", bufs=2,
                                              space="PSUM"))

        # ---- constants ----
        WallT_sb = cpool.tile([P, TW], bf16)
        nc.sync.dma_start(WallT_sb[:], WallT[:])
        vT8_sb = cpool.tile([2 * ED_DIM, 2 * H_HEADS], bf16)
        nc.sync.dma_start(vT8_sb[:], vT8[:])
        iota_rep = cpool.tile([P, K2 * P], bf16)  # value = col % 128
        nc.gpsimd.iota(iota_rep[:].rearrange("p (k u) -> p k u", u=P),
                       pattern=[[0, K2], [1, P]], base=0,
                       channel_multiplier=0,
                       allow_small_or_imprecise_dtypes=True)
        glo_sb = cpool.tile([P, NPAIR * 2 * KL * 8], i16)
        nc.sync.dma_start(glo_sb[:], glo16[:])
        ghi_sb = cpool.tile([P, NPAIR * 2 * KH * 8], i16)
        nc.sync.dma_start(ghi_sb[:], ghi16[:])
        dstloc_sb = cpool.tile([P, NPAIR * K2], bf16)
        nc.sync.dma_start(dstloc_sb[:], dstloc[:])
        invcnt_sb = cpool.tile([P, NWL], f32)
        nc.sync.dma_start(invcnt_sb[:], invcnt[:])

        # ---- phase T: node table = [xh | a_src | a_dst | 0 pad] ----
        XB = 8
        for g in range(math.ceil(NTT / XB)):
            t0 = g * XB
            nt = min(XB, NTT - t0)
            xt = xpool.tile([P, XB * P], bf16, tag="xt")
            nc.sync.dma_start(xt[:, :nt * P], xT[:, t0 * P:(t0 + nt) * P])
            tab = tabpool.tile([P, XB * TW], bf16, tag="tab")
            for t4 in range(0, nt, 4):
                n4 = min(4, nt - t4)
                ps = pst.tile([P, 4 * TW], f32)
                for t in range(t4, t4 + n4):
                    nc.tensor.matmul(
                        out=ps[:, (t - t4) * TW:(t - t4 + 1) * TW],
                        lhsT=xt[:, t * P:(t + 1) * P],
                        rhs=WallT_sb[:], start=True, stop=True)
                nc.vector.tensor_copy(
                    tab[:, t4 * TW:(t4 + n4) * TW], ps[:, :n4 * TW])
            nc.scalar.dma_start(
                out=tab3[:, t0:t0 + nt, :],
                in_=tab[:, :nt * TW].rearrange("p (t u) -> p t u", u=TW))

        # ---- own node rows, straight into SBUF (partition-id offset) ----
        selfall = cpool.tile([P, NWL * (HC + 8)], bf16)
        base = nc.sync.partition_id() * NWL
        nc.sync.dma_start(
            out=selfall[:].rearrange("p (w u) -> p w u", u=HC + 8),
            in_=tab3[:, bass.ds(base, NWL), 0:HC + 8])

        # ---- phase B: per-pair attention softmax + aggregation ----
        # block j of a pair belongs to window parity blk_win[j]:
        blk_win = [0] * KL + [1] * KL + [0] * KH + [1] * KH
        win_blocks = [[j for j in range(K2) if blk_win[j] == e]
                      for e in (0, 1)]
        for v in range(NPAIR):
            G = gpool.tile([P, K2 * TW], bf16, tag="G")
            Gv = G[:].rearrange("p (k u) -> p k u", u=TW)
            nc.gpsimd.dma_gather(
                out_ap=Gv[:, 0:2 * KL, :], in_ap=tableA[0:SPLIT, :],
                idxs_ap=glo_sb[:, v * 2 * KL * 8:(v + 1) * 2 * KL * 8],
                num_idxs=2 * KL * P, num_idxs_reg=2 * KL * P, elem_size=TW,
                single_packet=False, queue_num=(2 * v) % 4)
            nc.gpsimd.dma_gather(
                out_ap=Gv[:, 2 * KL:K2, :], in_ap=tableA[SPLIT:NT_PAD, :],
                idxs_ap=ghi_sb[:, v * 2 * KH * 8:(v + 1) * 2 * KH * 8],
                num_idxs=2 * KH * P, num_idxs_reg=2 * KH * P, elem_size=TW,
                single_packet=False, queue_num=(2 * v + 1) % 4)

            # transposed one-hot (lhsT of a_dst expansion): shipped from host
            St = stpool.tile([P, EPP], bf16, tag="St")
            nc.sync.dma_start(St[:], StA[:, v * EPP:(v + 1) * EPP])
            # one-hot S [edge, node]: one DVE op for the whole pair
            S = spool.tile([P, EPP], bf16, tag="S")
            nc.vector.tensor_tensor(
                out=S[:].rearrange("p (k u) -> p k u", u=P),
                in0=iota_rep[:].rearrange("p (k u) -> p k u", u=P),
                in1=dstloc_sb[:, v * K2:(v + 1) * K2].unsqueeze(2)
                .broadcast_to([P, K2, P]),
                op=mybir.AluOpType.is_equal)

            # a_dst(dst) per edge: St-block matmuls against own a_dst rows
            ps_adst = psad.tile([P, K2 * UH], f32)
            for j in range(K2):
                w = 2 * v + blk_win[j]
                nc.tensor.matmul(
                    out=ps_adst[:, j * UH:(j + 1) * UH],
                    lhsT=St[:, j * P:(j + 1) * P],
                    rhs=selfall[:, w * (HC + 8) + HC + 4:
                                w * (HC + 8) + HC + 8],
                    start=True, stop=True)

            # a_edge0 = edge_attr @ v.T for this pair (2 blocks per matmul)
            ea_ch = eapool.tile([2 * ED_DIM, K * P], bf16, tag="ea")
            nc.sync.dma_start(ea_ch[:], eaT2[:, v * K * P:(v + 1) * K * P])
            ps_e = pse.tile([P, K2 * UH], f32)
            for jj in range(K):
                nc.tensor.matmul(
                    out=ps_e[:, jj * 8:(jj + 1) * 8],
                    lhsT=ea_ch[:, jj * P:(jj + 1) * P],
                    rhs=vT8_sb[:], start=True, stop=True)

            # alpha = a_src(src) + a_dst(dst) + a_edge
            aw = wpool.tile([P, K2 * UH], f32, tag="aw")
            aw3 = aw[:].rearrange("p (k u) -> p k u", u=UH)
            nc.vector.tensor_tensor(
                out=aw3, in0=Gv[:, :, HC:HC + UH],
                in1=ps_adst[:].rearrange("p (k u) -> p k u", u=UH),
                op=mybir.AluOpType.add)
            nc.vector.tensor_tensor(
                out=aw[:], in0=aw[:], in1=ps_e[:], op=mybir.AluOpType.add)
            # lrelu(x) = slope*x + relu((1-slope)*x), then exp
            lrl = wpool.tile([P, K2 * UH], f32, tag="lrl")
            nc.scalar.activation(lrl[:], aw[:],
                                 mybir.ActivationFunctionType.Relu,
                                 scale=1.0 - NEG_SLOPE)
            nc.vector.scalar_tensor_tensor(
                out=lrl[:], in0=aw[:], scalar=NEG_SLOPE, in1=lrl[:],
                op0=mybir.AluOpType.mult, op1=mybir.AluOpType.add)

            # M = [expal * xh | expal | a_edge0] per block
            M = mpool.tile([P, K2 * MW], bf16, tag="M")
            M3 = M[:].rearrange("p (k u) -> p k u", u=MW)
            nc.scalar.activation(M3[:, :, HC:HC + UH],
                                 lrl[:].rearrange("p (k u) -> p k u", u=UH),
                                 mybir.ActivationFunctionType.Exp)
            nc.vector.tensor_copy(
                M3[:, :, HC + 4:HC + 8],
                ps_e[:].rearrange("p (k u) -> p k u", u=UH))
            expal_b = (M3[:, :, HC:HC + UH].unsqueeze(3)
                       .broadcast_to([P, K2, UH, C_OUT]))
            nc.vector.tensor_tensor(
                out=M3[:, :, 0:HC].rearrange("p k (h c) -> p k h c", c=C_OUT),
                in0=Gv[:, :, 0:HC].rearrange("p k (h c) -> p k h c", c=C_OUT),
                in1=expal_b, op=mybir.AluOpType.mult)

            # segment sums: one matmul per block, accumulated in PSUM;
            # both windows share one PSUM tile (disjoint column ranges)
            ps_agg = psa.tile([P, 2 * MW], f32)
            for e in (0, 1):
                blocks = win_blocks[e]
                for i, j in enumerate(blocks):
                    nc.tensor.matmul(
                        out=ps_agg[:, e * MW:(e + 1) * MW],
                        lhsT=S[:, j * P:(j + 1) * P],
                        rhs=M[:, j * MW:(j + 1) * MW],
                        start=(i == 0), stop=(i == len(blocks) - 1))

            # ---- window close: self-loop term + normalization ----
            for e in (0, 1):
                w = 2 * v + e
                agg = ps_agg[:, e * MW:(e + 1) * MW]
                selfr = selfall[:, w * (HC + 8):(w + 1) * (HC + 8)]
                lae = wpool.tile([P, 4], f32, tag=f"lae{e}")
                nc.vector.tensor_scalar(
                    out=lae[:], in0=agg[:, HC + 4:HC + 8],
                    scalar1=invcnt_sb[:, w:w + 1],
                    scalar2=None, op0=mybir.AluOpType.mult)
                asf = wpool.tile([P, 4], f32, tag=f"asf{e}")
                nc.vector.tensor_tensor(
                    out=asf[:], in0=selfr[:, HC:HC + 4],
                    in1=selfr[:, HC + 4:HC + 8], op=mybir.AluOpType.add)
                nc.vector.tensor_tensor(out=asf[:], in0=asf[:], in1=lae[:],
                                        op=mybir.AluOpType.add)
                es = wpool.tile([P, 4], f32, tag=f"es{e}")
                nc.scalar.activation(es[:], asf[:],
                                     mybir.ActivationFunctionType.Relu,
                                     scale=1.0 - NEG_SLOPE)
                nc.vector.scalar_tensor_tensor(
                    out=es[:], in0=asf[:], scalar=NEG_SLOPE, in1=es[:],
                    op0=mybir.AluOpType.mult, op1=mybir.AluOpType.add)
                nc.scalar.activation(es[:], es[:],
                                     mybir.ActivationFunctionType.Exp)
                # den = exp(alpha_self) + 1e-30 + sum_edges exp(alpha)
                den = wpool.tile([P, 4], f32, tag=f"den{e}")
                nc.vector.scalar_tensor_tensor(
                    out=den[:], in0=es[:], scalar=1e-30,
                    in1=agg[:, HC:HC + 4],
                    op0=mybir.AluOpType.add, op1=mybir.AluOpType.add)
                rec = wpool.tile([P, 4], f32, tag=f"rec{e}")
                nc.vector.reciprocal(rec[:], den[:])
                ot = opool.tile([P, HC], f32, tag=f"ot{e}")
                es_b = es[:].unsqueeze(2).broadcast_to([P, 4, C_OUT])
                nc.vector.tensor_tensor(
                    out=ot[:].rearrange("p (h c) -> p h c", c=C_OUT),
                    in0=selfr[:, 0:HC].rearrange("p (h c) -> p h c", c=C_OUT),
                    in1=es_b, op=mybir.AluOpType.mult)
                nc.vector.tensor_tensor(out=ot[:], in0=ot[:],
                                        in1=agg[:, 0:HC],
                                        op=mybir.AluOpType.add)
                rec_b = rec[:].unsqueeze(2).broadcast_to([P, 4, C_OUT])
                nc.vector.tensor_tensor(
                    out=ot[:].rearrange("p (h c) -> p h c", c=C_OUT),
                    in0=ot[:].rearrange("p (h c) -> p h c", c=C_OUT),
                    in1=rec_b, op=mybir.AluOpType.mult)
                nc.sync.dma_start(out[w * P:(w + 1) * P, :], ot[:])

    nc.compile()
    return nc


_NC_CACHE = {}


def _get_nc(cfg):
    k = cfg.key()
    if k not in _NC_CACHE:
        _NC_CACHE[k] = _build_nc(cfg)
    return _NC_CACHE[k]


def kernel(**inputs):
    x = np.asarray(inputs["x"], dtype=np.float32)
    ei = np.asarray(inputs["edge_index"])
    ea = np.asarray(inputs["edge_attr"], dtype=np.float32)
    W = np.asarray(inputs["W"], dtype=np.float32)
    W_edge = np.asarray(inputs["W_edge"], dtype=np.float32)
    att_src = np.asarray(inputs["att_src"], dtype=np.float32)
    att_dst = np.asarray(inputs["att_dst"], dtype=np.float32)
    att_edge = np.asarray(inputs["att_edge"], dtype=np.float32)
    bias = np.asarray(inputs["bias"], dtype=np.float32)

    src = ei[0].astype(np.int64)
    dst = ei[1].astype(np.int64)
    WallT, vT8 = _fold_weights(W, W_edge, att_src, att_dst, att_edge)

    cfg, in_maps, meta = _prep(x, src, dst, ea, WallT, vT8)
    nc = _get_nc(cfg)

    from concourse.bass_utils import run_bass_kernel_spmd
    res = run_bass_kernel_spmd(nc, in_maps, core_ids=list(range(NCORES)),
                               trace=TRACE)
    if TRACE:
        global LAST_RESULT
        LAST_RESULT = res

    out_ws = np.concatenate([res.results[c]["out"] for c in range(NCORES)],
                            axis=0)  # window-space [n_windows*P, HC]
    out = out_ws[meta["winpos"]]
    return (out + bias[None, :]).astype(np.float32)
